# revision 1
# baseline (speedup 1.0000x reference)
"""Causal self-attention (B=4, S=2048, E=1024, D=128, single head) on 8 TRN2 cores.

Sharding: core c = 2*b + h handles batch b; the two cores of a pair split the
causal key range by k-tile parity (h=0 even 128-row k-tiles, h=1 odd). All 8
cores run the *same* instruction stream; per-core differences live in DRAM
data only:
  - xt [1024, 2048] fp16: x[b].T with 128-col s-tiles stored in "perm order"
    (position p holds global tile p^h), so EVEN positions are always the
    core's own-parity tiles. K/V projection reads even positions via a
    strided AP; Q projection reads all positions in storage order.
  - mask [128, 256] fp16: additive score mask for the two diagonal-region
    k-tiles of every q-block ([tri | 0] for h=0, [tri | -6e4] for h=1).

Attention runs over four contiguous 512-col q-blocks (perm order ~ natural
order up to intra-pair swaps). Block J attends local k-tiles i=0..2J+1 with
an exact-causal trapezoid: tile i=2J+1 streams only columns [256:512); tiles
i=2J and 2J+1 get the additive mask (DVE, off the PE) before exp. Softmax
denominators come from accumulating exp tiles into ACC (DVE fp16 2x-rate
adds) and ONE rank-1 ones^T @ ACC matmul per block, instead of one per
k-tile. PSUM->SBUF output staging copies run on the ACT engine (DVE was the
busier engine). All PE operands are fp16; PSUM stays fp32. Per-core PE
streaming ~54k nominal cycles vs ~79k for the f32r/mask-matmul/per-tile-sums
baseline - and fp16 moving operands stream ~2 cols/cycle on HW, which f32r
does not, so the PE-bound steady state roughly halves on top of that.

Each core emits unnormalized PV partials (pvt [128 d, 2048 q perm]) and
denominators (sums [1, 2048]); the host un-permutes and combines the pair:
  out[b] = ((pv0 + pv1) / (s0 + s1)).T

Measured (reps-delta wall-clock, see prof.py): rel err 5.4e-4; ~12-13
us/iter steady state in the device's fast phase (best rounds 9.3-11.4 us;
the shared device stretches ~25% in slow phases) vs ~30.6 us for the staged
baseline under the same methodology (grader-reported baseline: 26.6 us).
Compute-bound: a probe build that skips the 4 MB xt upload still takes
~12.4 us/iter, i.e. PE streaming is ~90% of the critical path.
Phase-controlled A/Bs settled the emission structure: sequential per-chunk
emission beats software-pipelining (attention one group behind projection);
one DMA stream per hwdge ring beats splitting xt across SP+ACT; in-block
tile order [first, masked, ..., plain-last] is kept. The pipeline= and
split_rings= flags on _build_module reproduce the losing variants.
"""

import os

os.environ.setdefault("MYCRO_LOCAL_CACHE", "1")

import numpy as np

B, S, E, D = 4, 2048, 1024, 128
P = 128
NT = S // P          # 16 global s-tiles per batch
LT = NT // 2         # 8 local (per-core) k-tiles
NQB = 4              # 512-wide query blocks
QBW = 512
NEB = E // P         # 8 e-tiles
SCALE = 1.0 / float(np.sqrt(D))
NEG = -60000.0       # fp16-representable; exp underflows to 0 in fp32

TRACE = False        # set by test.py for profiling runs
TRACE_KW = {}
PROBE_NO_XT = False  # timing probe: skip xt DMA (results wrong; perf only)

_CACHE = {}


def _build_module(reps=1, pipeline=False, split_rings=False, fp8qk=False, packed_kv=False):
    from contextlib import ExitStack

    import concourse.bacc as bacc
    import concourse.mybir as mybir
    import concourse.tile as tile

    f32 = mybir.dt.float32
    f16 = mybir.dt.float16
    f8 = mybir.dt.float8e4

    nc = bacc.Bacc("TRN2", target_bir_lowering=False, debug=False, num_devices=8)

    if fp8qk:
        # e = 256*p4 + 2*ki + ko interleave for DoubleRow (K=256 per pass)
        x8_d = nc.dram_tensor("x8", [P, 4, 2, S], f8, kind="ExternalInput").ap()
        xtv_d = nc.dram_tensor("xtv", [E, S // 2], f16, kind="ExternalInput").ap()
        wq8_d = nc.dram_tensor("wq8", [P, 4, 2, D], f8, kind="ExternalInput").ap()
        wk8_d = nc.dram_tensor("wk8", [P, 4, 2, D], f8, kind="ExternalInput").ap()
    else:
        xt_d = nc.dram_tensor("xt", [E, S], f16, kind="ExternalInput").ap()
        if packed_kv:
            xtv_d = nc.dram_tensor("xtv", [E, S // 2], f16, kind="ExternalInput").ap()
    wq_d = nc.dram_tensor("wq", [E, D], f16, kind="ExternalInput").ap()
    wk_d = nc.dram_tensor("wk", [E, D], f16, kind="ExternalInput").ap()
    wv_d = nc.dram_tensor("wv", [E, D], f16, kind="ExternalInput").ap()
    bq_d = nc.dram_tensor("bq", [D], f32, kind="ExternalInput").ap()  # pre-scaled
    bk_d = nc.dram_tensor("bk", [D], f32, kind="ExternalInput").ap()
    bv_d = nc.dram_tensor("bv", [D], f32, kind="ExternalInput").ap()
    mask_d = nc.dram_tensor("mask", [P, 2 * P], f16, kind="ExternalInput").ap()
    ones_d = nc.dram_tensor("ones", [P, 1], f16, kind="ExternalInput").ap()
    ident_d = nc.dram_tensor("ident", [P, P], f16, kind="ExternalInput").ap()
    pvt_d = nc.dram_tensor("pvt", [D, S], f32, kind="ExternalOutput").ap()
    sums_d = nc.dram_tensor("sums", [1, S], f32, kind="ExternalOutput").ap()

    with tile.TileContext(nc) as tc, ExitStack() as ctx:
        singles = ctx.enter_context(tc.tile_pool(name="singles", bufs=1))
        ppool = ctx.enter_context(tc.tile_pool(name="ppool", bufs=4))
        apool = ctx.enter_context(tc.tile_pool(name="apool", bufs=2))
        proj_ps = ctx.enter_context(tc.tile_pool(name="proj_ps", bufs=1, space="PSUM"))
        tr_ps = proj_ps
        sc_ps = ctx.enter_context(tc.tile_pool(name="sc_ps", bufs=3, space="PSUM"))
        pv_ps = ctx.enter_context(tc.tile_pool(name="pv_ps", bufs=1, space="PSUM"))
        sum_ps = ctx.enter_context(tc.tile_pool(name="sum_ps", bufs=1, space="PSUM"))

        # ---- constants (ACT HWDGE ring; xt stream owns the SP ring) ----
        w_sb = {}
        w_names = (("wv", wv_d),) if fp8qk else (
            ("wk", wk_d), ("wv", wv_d), ("wq", wq_d))
        for name, dram in w_names:
            t = singles.tile([P, NEB, D], f16, tag=f"w_{name}")
            nc.scalar.dma_start(t[:], dram.rearrange("(o p) d -> p o d", p=P))
            w_sb[name] = t
        w8 = {}
        if fp8qk:
            for name, dram in (("wq8", wq8_d), ("wk8", wk8_d)):
                t = singles.tile([P, 4, 2, D], f8, tag=f"w_{name}")
                nc.scalar.dma_start(t[:], dram[:])
                w8[name] = t
        b_sb = {}
        for name, dram in (("bq", bq_d), ("bk", bk_d), ("bv", bv_d)):
            t = singles.tile([P, 1], f32, tag=f"b_{name}")
            nc.scalar.dma_start(t[:], dram.rearrange("(p one) -> p one", one=1))
            b_sb[name] = t
        mask_sb = singles.tile([P, 2 * P], f16, tag="mask")
        nc.scalar.dma_start(mask_sb[:], mask_d[:])
        ones = singles.tile([P, 1], f16, tag="ones")
        nc.scalar.dma_start(ones[:], ones_d[:])
        ident = singles.tile([P, P], f16, tag="ident")
        nc.scalar.dma_start(ident[:], ident_d[:])

        # ---- persistent activations ----
        # xt resident in perm order: [e-part, e-tile, pair, parity-pos, col]
        if fp8qk:
            x8_sb = singles.tile([P, 4, 2, LT, 2, P], f8, tag="x8_sb")
            xtv_sb = singles.tile([P, NEB, LT, P], f16, tag="xtv_sb")
        else:
            xt_sb = singles.tile([P, NEB, LT, 2, P], f16, tag="xt_sb")
            if packed_kv:
                xtv_sb = singles.tile([P, NEB, LT, P], f16, tag="xtv_sb")
        kt = singles.tile([P, LT, P], f16, tag="kt")      # K^T  [d, lt, k]
        vt = singles.tile([P, LT, P], f16, tag="vt")      # V^T  [d, lt, k]
        vn = singles.tile([P, LT, D], f16, tag="vn")      # V natural [k, lt, d]
        qt = singles.tile([P, NT, P], f16, tag="qt")      # Q^T [d, pos, q]
        pvt_sb = singles.tile([D, S], f32, tag="pvt_sb")
        sums_sb = singles.tile([1, S], f32, tag="sums_sb")

        ktv = kt.rearrange("p lt k -> p (lt k)")
        vtv = vt.rearrange("p lt k -> p (lt k)")

        def load_chunk(J, rep=0):
            """DMA xt columns [512J, 512J+512) (positions 4J..4J+3)."""
            if PROBE_NO_XT and rep > 0:
                return
            if fp8qk:
                nc.sync.dma_start(
                    x8_sb[:, :, :, 2 * J : 2 * J + 2, :, :],
                    x8_d[:, :, :, J * QBW : (J + 1) * QBW],
                )
                for eo in range(NEB):
                    nc.sync.dma_start(
                        xtv_sb[:, eo, 2 * J : 2 * J + 2, :],
                        xtv_d[eo * P : (eo + 1) * P, J * 2 * P : (J + 1) * 2 * P],
                    )
                return
            for eo in range(NEB):
                eng = nc.sync if (eo % 2 == 0 or not split_rings) else nc.scalar
                eng.dma_start(
                    xt_sb[:, eo, 2 * J : 2 * J + 2, :, :],
                    xt_d[eo * P : (eo + 1) * P, J * QBW : (J + 1) * QBW],
                )
                if packed_kv:
                    nc.scalar.dma_start(
                        xtv_sb[:, eo, 2 * J : 2 * J + 2, :],
                        xtv_d[eo * P : (eo + 1) * P, J * 2 * P : (J + 1) * 2 * P],
                    )

        def proj_chunk(J):
            """K/V for local k-tiles {2J, 2J+1} + Q for block J."""
            if fp8qk:
                # K: DoubleRow over full 512 perm cols (odd positions wasted,
                # but full-width keeps the rhs AP contiguous); keep evens.
                ps = proj_ps.tile([P, 2, 2, P], f32, tag="ps_kv")
                for p4 in range(4):
                    nc.tensor.matmul(
                        ps[:],
                        w8["wk8"][:, p4, :, :],
                        x8_sb[:, p4, :, 2 * J : 2 * J + 2, :, :],
                        start=(p4 == 0),
                        stop=(p4 == 3),
                        perf_mode=mybir.MatmulPerfMode.DoubleRow,
                    )
                nc.vector.tensor_scalar_add(
                    ktv[:, J * 2 * P : (J + 1) * 2 * P], ps[:, :, 0, :],
                    b_sb["bk"][:],
                )
                psv = proj_ps.tile([P, QBW], f32, tag="ps_kv")
                for eo in range(NEB):
                    nc.tensor.matmul(
                        psv[:, : 2 * P],
                        w_sb["wv"][:, eo, :],
                        xtv_sb[:, eo, 2 * J : 2 * J + 2, :],
                        start=(eo == 0),
                        stop=(eo == NEB - 1),
                    )
                nc.vector.tensor_scalar_add(
                    vtv[:, J * 2 * P : (J + 1) * 2 * P], psv[:, : 2 * P],
                    b_sb["bv"][:],
                )
                ps = proj_ps.tile([P, QBW], f32, tag="ps_q")
                for p4 in range(4):
                    nc.tensor.matmul(
                        ps[:],
                        w8["wq8"][:, p4, :, :],
                        x8_sb[:, p4, :, 2 * J : 2 * J + 2, :, :],
                        start=(p4 == 0),
                        stop=(p4 == 3),
                        perf_mode=mybir.MatmulPerfMode.DoubleRow,
                    )
                qv = qt.rearrange("p t q -> p (t q)")
                nc.vector.tensor_scalar(
                    qv[:, J * QBW : (J + 1) * QBW],
                    ps[:],
                    SCALE,
                    b_sb["bq"][:],
                    mybir.AluOpType.mult,
                    mybir.AluOpType.add,
                )
                return
            for name, dstv, bias in (("wk", ktv, "bk"), ("wv", vtv, "bv")):
                ps = proj_ps.tile([P, QBW], f32, tag="ps_kv")
                for eo in range(NEB):
                    rhs = (
                        xtv_sb[:, eo, 2 * J : 2 * J + 2, :]
                        if packed_kv
                        else xt_sb[:, eo, 2 * J : 2 * J + 2, 0, :]
                    )
                    nc.tensor.matmul(
                        ps[:, : 2 * P],
                        w_sb[name][:, eo, :],
                        rhs,
                        start=(eo == 0),
                        stop=(eo == NEB - 1),
                    )
                nc.vector.tensor_scalar_add(
                    dstv[:, J * 2 * P : (J + 1) * 2 * P], ps[:, : 2 * P], b_sb[bias][:]
                )
            ps = proj_ps.tile([P, QBW], f32, tag="ps_q")
            for eo in range(NEB):
                nc.tensor.matmul(
                    ps[:],
                    w_sb["wq"][:, eo, :],
                    xt_sb[:, eo, 2 * J : 2 * J + 2, :, :],
                    start=(eo == 0),
                    stop=(eo == NEB - 1),
                )
            qv = qt.rearrange("p t q -> p (t q)")
            nc.vector.tensor_scalar(
                qv[:, J * QBW : (J + 1) * QBW],
                ps[:],
                SCALE,
                b_sb["bq"][:],
                mybir.AluOpType.mult,
                mybir.AluOpType.add,
            )

        def v_transpose(lt):
            ps = tr_ps.tile([P, P], f16, tag="tr")
            nc.tensor.transpose(ps[:], vt[:, lt, :], ident[:])
            nc.vector.tensor_copy(out=vn[:, lt, :], in_=ps[:])

        def attention_blk(J):
            """Block J: q-cols [512J, 512J+512), local k-tiles 0..2J+1."""
            nlt = 2 * J + 2
            col0 = J * QBW
            pv = pv_ps.tile([P, QBW], f32, tag="pv")
            acc = apool.tile([P, QBW], f16, tag="acc")
            # emission order: full-width i=0 first (clears PSUM over the
            # whole block), then the masked tiles (their DVE-mask + exp
            # latency hides behind the remaining scores matmuls), ending on
            # a plain tile so only one exp latency is exposed at block end.
            if J == 0:
                order = [0, 1]
            else:
                order = [0, 2 * J, 2 * J + 1] + list(range(1, 2 * J))
            for idx, i in enumerate(order):
                c0 = 2 * P if i == 2 * J + 1 else 0
                sc = sc_ps.tile([P, QBW], f32, tag="sc")
                nc.tensor.matmul(
                    sc[:, c0:],
                    kt[:, i, :],
                    qt[:, 4 * J + c0 // P : 4 * J + 4, :],
                    start=True,
                    stop=True,
                )
                if i >= 2 * J:
                    nc.vector.tensor_tensor(
                        out=sc[:, c0 : c0 + 2 * P],
                        in0=sc[:, c0 : c0 + 2 * P],
                        in1=mask_sb[:],
                        op=mybir.AluOpType.add,
                    )
                if idx == 0:
                    psrc = acc
                    nc.scalar.activation(
                        acc[:], sc[:], mybir.ActivationFunctionType.Exp
                    )
                else:
                    psrc = ppool.tile([P, QBW], f16, tag="p")
                    nc.scalar.activation(
                        psrc[:, c0:], sc[:, c0:], mybir.ActivationFunctionType.Exp
                    )
                    nc.vector.tensor_tensor(
                        out=acc[:, c0:],
                        in0=acc[:, c0:],
                        in1=psrc[:, c0:],
                        op=mybir.AluOpType.add,
                    )
                nc.tensor.matmul(
                    pv[:, c0:],
                    vn[:, i, :],
                    psrc[:, c0:],
                    start=(idx == 0),
                    stop=(idx == nlt - 1),
                    skip_group_check=True,
                )
            return pv, acc

        def finish_blk(J, pv, acc):
            """Block J tail: denominator matmul, staging copies, output DMA.
            Emitted one chunk-group after attention_blk(J) so the PE has
            projection work queued while ACT/DVE drain the block tail."""
            col0 = J * QBW
            sm = sum_ps.tile([1, QBW], f32, tag="sm")
            nc.tensor.matmul(sm[:], ones[:], acc[:], start=True, stop=True)
            nc.scalar.copy(out=pvt_sb[:, col0 : col0 + QBW], in_=pv[:])
            nc.scalar.copy(out=sums_sb[:, col0 : col0 + QBW], in_=sm[:])
            out_eng = nc.gpsimd
            out_eng.dma_start(
                pvt_d[:, col0 : col0 + QBW], pvt_sb[:, col0 : col0 + QBW]
            )
            out_eng.dma_start(
                sums_d[:, col0 : col0 + QBW], sums_sb[:, col0 : col0 + QBW]
            )

        # ---- emission order (priority hint for the scheduler) ----
        # Software pipeline: group k emits chunk-k%4's DMA + projections,
        # then finish(k-2), then attention(k-1). Attention never sits at the
        # head of the PE queue behind its own block tail, and each block's
        # exposed exp/acc latency is covered by the next chunk's projections
        # (across the rep boundary too).
        pend_attn = None   # J of attention not yet emitted
        pend_fin = None    # (J, pv, acc) of finish not yet emitted
        for _rep in range(reps):
            for J in range(NQB):
                load_chunk(J, _rep)
                proj_chunk(J)
                v_transpose(2 * J)
                v_transpose(2 * J + 1)
                if not pipeline:
                    finish_blk(J, *attention_blk(J))
                    continue
                if pend_fin is not None:
                    finish_blk(*pend_fin)
                    pend_fin = None
                if pend_attn is not None:
                    pend_fin = (pend_attn, *attention_blk(pend_attn))
                pend_attn = J
        # drain the pipeline tail
        if pend_fin is not None:
            finish_blk(*pend_fin)
        if pend_attn is not None:
            finish_blk(pend_attn, *attention_blk(pend_attn))

    nc.compile()
    return nc


def _get_module(reps=1, pipeline=False, split_rings=False, fp8qk=False,
                packed_kv=False):
    key = ("nc", reps, pipeline, split_rings, fp8qk, packed_kv)
    if key not in _CACHE:
        _CACHE[key] = _build_module(reps, pipeline, split_rings, fp8qk, packed_kv)
    return _CACHE[key]


def _host_prep(x, Wq, bq, Wk, bk, Wv, bv, fp8qk=False):
    """Build the 8 per-core input maps plus per-core q-column permutations.
    fp8qk=True adds the interleaved fp8 tensors for the (rejected) DoubleRow
    projection variant -- see the fp8qk flag on _build_module."""
    x = np.asarray(x, dtype=np.float32)
    tri = np.where(
        np.arange(P)[None, :] >= np.arange(P)[:, None], 0.0, NEG
    ).astype(np.float16)
    in_maps = []
    perms = []
    for c in range(8):
        b, h = divmod(c, 2)
        xt3 = np.ascontiguousarray(x[b].T).reshape(E, NT, P)
        # perm order: position p holds global tile p^h
        pos = np.arange(NT) ^ h
        xt_perm = np.ascontiguousarray(xt3[:, pos, :].reshape(E, S)).astype(
            np.float16
        )
        mask = np.concatenate(
            [tri, np.full((P, P), 0.0 if h == 0 else NEG, np.float16)], axis=1
        )
        extra = {
            "xtv": np.ascontiguousarray(
                xt_perm.reshape(E, NT, P)[:, 0::2, :].reshape(E, S // 2)
            )
        }
        if fp8qk:
            import ml_dtypes
            f8 = ml_dtypes.float8_e4m3
            x8 = np.ascontiguousarray(
                xt_perm.astype(np.float32).reshape(4, P, 2, S).transpose(1, 0, 2, 3)
            )
            w8q = np.asarray(Wq, np.float32).reshape(4, P, 2, D).transpose(1, 0, 2, 3)
            w8k = np.asarray(Wk, np.float32).reshape(4, P, 2, D).transpose(1, 0, 2, 3)
            extra.update(
                {
                    "x8": x8.astype(f8),
                    "wq8": np.ascontiguousarray(w8q).astype(f8),
                    "wk8": np.ascontiguousarray(w8k).astype(f8),
                }
            )
        in_maps.append(
            {
                **extra,
                "xt": xt_perm,
                "wq": np.asarray(Wq, np.float16),
                "wk": np.asarray(Wk, np.float16),
                "wv": np.asarray(Wv, np.float16),
                "bq": np.asarray(bq, np.float32) * np.float32(SCALE),
                "bk": np.asarray(bk, np.float32),
                "bv": np.asarray(bv, np.float32),
                "mask": np.ascontiguousarray(mask),
                "ones": np.ones((P, 1), dtype=np.float16),
                "ident": np.eye(P, dtype=np.float16),
            }
        )
        # storage col -> global q row (position tile p holds global tile p^h)
        perm = np.empty(S, dtype=np.int64)
        for t in range(NT):
            perm[t * P : (t + 1) * P] = (t ^ h) * P + np.arange(P)
        perms.append(perm)
    return in_maps, perms


def kernel(x, Wq, bq, Wk, bk, Wv, bv):
    from concourse.bass_utils import run_bass_kernel_spmd

    nc = _get_module()
    in_maps, perms = _host_prep(x, Wq, bq, Wk, bk, Wv, bv)
    res = run_bass_kernel_spmd(
        nc,
        in_maps,
        core_ids=list(range(8)),
        trace=TRACE,
        **TRACE_KW,
    )
    _CACHE["last_result"] = res

    out = np.empty((B, S, D), dtype=np.float32)
    for b in range(B):
        r0, r1 = res.results[2 * b], res.results[2 * b + 1]
        pv = np.zeros((D, S), dtype=np.float64)
        sm = np.zeros((S,), dtype=np.float64)
        for r, perm in ((r0, perms[2 * b]), (r1, perms[2 * b + 1])):
            pv[:, perm] += r["pvt"].astype(np.float64)
            sm[perm] += r["sums"][0].astype(np.float64)
        out[b] = (pv / sm[None, :]).T.astype(np.float32)
    return out



# revision 6
# speedup vs baseline: 28.6941x; 28.6941x over previous
"""Causal self-attention (B=4, S=2048, E=1024, D=128, single head) on 8 TRN2 cores.

Sharding: core c = 2*b + h handles batch b; the two cores of a pair split the
causal key range by k-tile parity (h=0 even 128-row k-tiles, h=1 odd). All 8
cores run the *same* instruction stream; per-core differences live in DRAM
data only:
  - xt [1024, 2048] fp16: x[b].T with 128-col s-tiles stored in "perm order"
    (position p holds global tile p^h), so EVEN positions are always the
    core's own-parity tiles. K/V projection reads even positions via a
    strided AP; Q projection reads all positions in storage order.
  - mask [128, 256] fp16: additive score mask for the two diagonal-region
    k-tiles of every q-block ([tri | 0] for h=0, [tri | -6e4] for h=1).

Math shortcuts vs the reference:
  - K bias dropped entirely: softmax(q.(k+bk)) == softmax(q.k) since the
    q.bk term is constant across keys for a fixed query row.
  - V bias moved to the host: rows of normalized attention sum to 1, so
    out = (pv/sums) + bv exactly.
  - V is projected directly in NATURAL orientation (stationary = 128x128
    x-block, moving = Wv e-tile), killing the 8 PE transposes (~275ns each)
    and the identity tensor.

Engine split (Pool/gpsimd has no PSUM port): PE matmuls; ACT exp; DVE mask
adds + PSUM->SBUF copies (K, Q-bias-scale, V-natural, pv/sums staging); Pool
the SBUF fp16 exp-accumulation adds + output DMA ring. PV output is stored
fp16 (pvt [128 d, 2048 q perm]); denominators stay f32.

Each core emits unnormalized PV partials and denominators (sums [1, 2048]);
the host un-permutes, combines the pair, and adds bv:
  out[b] = ((pv0 + pv1) / (s0 + s1)).T + bv
"""

import os

os.environ.setdefault("MYCRO_LOCAL_CACHE", "1")

import numpy as np

B, S, E, D = 4, 2048, 1024, 128
P = 128
NT = S // P          # 16 global s-tiles per batch
LT = NT // 2         # 8 local (per-core) k-tiles
NQB = 4              # 512-wide query blocks
QBW = 512
NEB = E // P         # 8 e-tiles
SCALE = 1.0 / float(np.sqrt(D))
NEG = -60000.0       # fp16-representable; exp underflows to 0 in fp32

TRACE = False        # set by test.py for profiling runs
TRACE_KW = {}
PROBE_NO_XT = False  # timing probe: skip xt DMA (results wrong; perf only)

_CACHE = {}


def _build_module(reps=1, acc_pool_split=0):
    """acc_pool_split: exp-accumulation adds with idx <= split go to DVE,
    the rest to Pool. 0 = all on Pool."""
    from contextlib import ExitStack

    import concourse.bacc as bacc
    import concourse.mybir as mybir
    import concourse.tile as tile

    f32 = mybir.dt.float32
    f16 = mybir.dt.float16

    nc = bacc.Bacc("TRN2", target_bir_lowering=False, debug=False, num_devices=8)

    xt_d = nc.dram_tensor("xt", [E, S], f16, kind="ExternalInput").ap()
    wq_d = nc.dram_tensor("wq", [E, D], f16, kind="ExternalInput").ap()
    wk_d = nc.dram_tensor("wk", [E, D], f16, kind="ExternalInput").ap()
    wv_d = nc.dram_tensor("wv", [E, D], f16, kind="ExternalInput").ap()
    bq_d = nc.dram_tensor("bq", [D], f32, kind="ExternalInput").ap()  # pre-scaled
    mask_d = nc.dram_tensor("mask", [P, 2 * P], f16, kind="ExternalInput").ap()
    ones_d = nc.dram_tensor("ones", [P, 1], f16, kind="ExternalInput").ap()
    pvt_d = nc.dram_tensor("pvt", [D, S], f16, kind="ExternalOutput").ap()
    sums_d = nc.dram_tensor("sums", [1, S], f32, kind="ExternalOutput").ap()

    with tile.TileContext(nc) as tc, ExitStack() as ctx:
        singles = ctx.enter_context(tc.tile_pool(name="singles", bufs=1))
        ppool = ctx.enter_context(tc.tile_pool(name="ppool", bufs=4))
        apool = ctx.enter_context(tc.tile_pool(name="apool", bufs=2))
        proj_ps = ctx.enter_context(tc.tile_pool(name="proj_ps", bufs=1, space="PSUM"))
        v_ps = ctx.enter_context(tc.tile_pool(name="v_ps", bufs=2, space="PSUM"))
        sc_ps = ctx.enter_context(tc.tile_pool(name="sc_ps", bufs=3, space="PSUM"))
        pv_ps = ctx.enter_context(tc.tile_pool(name="pv_ps", bufs=1, space="PSUM"))
        sum_ps = ctx.enter_context(tc.tile_pool(name="sum_ps", bufs=1, space="PSUM"))

        # ---- constants (ACT HWDGE ring; xt stream owns the SP ring) ----
        w_sb = {}
        for name, dram in (("wk", wk_d), ("wv", wv_d), ("wq", wq_d)):
            t = singles.tile([P, NEB, D], f16, tag=f"w_{name}")
            nc.scalar.dma_start(t[:], dram.rearrange("(o p) d -> p o d", p=P))
            w_sb[name] = t
        bq_sb = singles.tile([P, 1], f32, tag="b_bq")
        nc.scalar.dma_start(bq_sb[:], bq_d.rearrange("(p one) -> p one", one=1))
        mask_sb = singles.tile([P, 2 * P], f16, tag="mask")
        nc.scalar.dma_start(mask_sb[:], mask_d[:])
        ones = singles.tile([P, 1], f16, tag="ones")
        nc.scalar.dma_start(ones[:], ones_d[:])

        # ---- persistent activations ----
        # xt resident in perm order: [e-part, e-tile, pair, parity-pos, col]
        xt_sb = singles.tile([P, NEB, LT, 2, P], f16, tag="xt_sb")
        kt = singles.tile([P, LT, P], f16, tag="kt")      # K^T  [d, lt, k]
        vn = singles.tile([P, LT, D], f16, tag="vn")      # V natural [k, lt, d]
        qt = singles.tile([P, NT, P], f16, tag="qt")      # Q^T [d, pos, q]
        pvt_sb = singles.tile([D, S], f16, tag="pvt_sb")
        sums_sb = singles.tile([1, S], f32, tag="sums_sb")

        ktv = kt.rearrange("p lt k -> p (lt k)")

        def load_chunk(J, rep=0):
            """DMA xt columns [512J, 512J+512) (positions 4J..4J+3)."""
            if PROBE_NO_XT and rep > 0:
                return
            for eo in range(NEB):
                nc.sync.dma_start(
                    xt_sb[:, eo, 2 * J : 2 * J + 2, :, :],
                    xt_d[eo * P : (eo + 1) * P, J * QBW : (J + 1) * QBW],
                )

        def proj_chunk(J):
            """K/V for local k-tiles {2J, 2J+1} + Q for block J."""
            # K^T: stationary Wk e-tile, moving x evens; no bias (see docstring)
            ps = proj_ps.tile([P, QBW], f32, tag="ps_kq")
            for eo in range(NEB):
                nc.tensor.matmul(
                    ps[:, : 2 * P],
                    w_sb["wk"][:, eo, :],
                    xt_sb[:, eo, 2 * J : 2 * J + 2, 0, :],
                    start=(eo == 0),
                    stop=(eo == NEB - 1),
                )
            nc.vector.tensor_copy(
                out=ktv[:, J * 2 * P : (J + 1) * 2 * P], in_=ps[:, : 2 * P]
            )
            # V natural: stationary x-block [e, s-tile], moving Wv e-tile
            for lt in (2 * J, 2 * J + 1):
                vps = v_ps.tile([P, D], f32, tag="ps_v")
                for eo in range(NEB):
                    nc.tensor.matmul(
                        vps[:],
                        xt_sb[:, eo, lt, 0, :],
                        w_sb["wv"][:, eo, :],
                        start=(eo == 0),
                        stop=(eo == NEB - 1),
                    )
                nc.vector.tensor_copy(out=vn[:, lt, :], in_=vps[:])
            # Q^T over all 4 positions of the chunk
            ps = proj_ps.tile([P, QBW], f32, tag="ps_kq")
            for eo in range(NEB):
                nc.tensor.matmul(
                    ps[:],
                    w_sb["wq"][:, eo, :],
                    xt_sb[:, eo, 2 * J : 2 * J + 2, :, :],
                    start=(eo == 0),
                    stop=(eo == NEB - 1),
                )
            qv = qt.rearrange("p t q -> p (t q)")
            nc.vector.tensor_scalar(
                qv[:, J * QBW : (J + 1) * QBW],
                ps[:],
                SCALE,
                bq_sb[:],
                mybir.AluOpType.mult,
                mybir.AluOpType.add,
            )

        def attention_blk(J):
            """Block J: q-cols [512J, 512J+512), local k-tiles 0..2J+1."""
            nlt = 2 * J + 2
            pv = pv_ps.tile([P, QBW], f32, tag="pv")
            acc = apool.tile([P, QBW], f16, tag="acc")
            # emission order: full-width i=0 first (clears PSUM over the
            # whole block), then the masked tiles (their DVE-mask + exp
            # latency hides behind the remaining scores matmuls), ending on
            # a plain tile so only one exp latency is exposed at block end.
            if J == 0:
                order = [0, 1]
            else:
                order = [0, 2 * J, 2 * J + 1] + list(range(1, 2 * J))
            for idx, i in enumerate(order):
                c0 = 2 * P if i == 2 * J + 1 else 0
                sc = sc_ps.tile([P, QBW], f32, tag="sc")
                nc.tensor.matmul(
                    sc[:, c0:],
                    kt[:, i, :],
                    qt[:, 4 * J + c0 // P : 4 * J + 4, :],
                    start=True,
                    stop=True,
                )
                if i >= 2 * J:
                    nc.vector.tensor_tensor(
                        out=sc[:, c0 : c0 + 2 * P],
                        in0=sc[:, c0 : c0 + 2 * P],
                        in1=mask_sb[:],
                        op=mybir.AluOpType.add,
                    )
                if idx == 0:
                    psrc = acc
                    nc.scalar.activation(
                        acc[:], sc[:], mybir.ActivationFunctionType.Exp
                    )
                else:
                    psrc = ppool.tile([P, QBW], f16, tag="p")
                    nc.scalar.activation(
                        psrc[:, c0:], sc[:, c0:], mybir.ActivationFunctionType.Exp
                    )
                    add_eng = nc.vector if idx <= acc_pool_split else nc.gpsimd
                    add_eng.tensor_tensor(
                        out=acc[:, c0:],
                        in0=acc[:, c0:],
                        in1=psrc[:, c0:],
                        op=mybir.AluOpType.add,
                    )
                nc.tensor.matmul(
                    pv[:, c0:],
                    vn[:, i, :],
                    psrc[:, c0:],
                    start=(idx == 0),
                    stop=(idx == nlt - 1),
                    skip_group_check=True,
                )
            return pv, acc

        def finish_blk(J, pv, acc):
            """Block J tail: denominator matmul, staging copies, output DMA."""
            col0 = J * QBW
            sm = sum_ps.tile([1, QBW], f32, tag="sm")
            nc.tensor.matmul(sm[:], ones[:], acc[:], start=True, stop=True)
            nc.vector.tensor_copy(out=pvt_sb[:, col0 : col0 + QBW], in_=pv[:])
            nc.vector.tensor_copy(out=sums_sb[:, col0 : col0 + QBW], in_=sm[:])
            out_eng = nc.gpsimd
            out_eng.dma_start(
                pvt_d[:, col0 : col0 + QBW], pvt_sb[:, col0 : col0 + QBW]
            )
            out_eng.dma_start(
                sums_d[:, col0 : col0 + QBW], sums_sb[:, col0 : col0 + QBW]
            )

        for _rep in range(reps):
            for J in range(NQB):
                load_chunk(J, _rep)
                proj_chunk(J)
                finish_blk(J, *attention_blk(J))

    nc.compile()
    return nc


def _get_module(reps=1, **kw):
    key = ("nc", reps, tuple(sorted(kw.items())))
    if key not in _CACHE:
        _CACHE[key] = _build_module(reps, **kw)
    return _CACHE[key]


def _host_prep(x, Wq, bq, Wk, bk, Wv, bv):
    """Build the 8 per-core input maps plus per-core q-column permutations."""
    x = np.asarray(x, dtype=np.float32)
    tri = np.where(
        np.arange(P)[None, :] >= np.arange(P)[:, None], 0.0, NEG
    ).astype(np.float16)
    in_maps = []
    perms = []
    for c in range(8):
        b, h = divmod(c, 2)
        xt3 = np.ascontiguousarray(x[b].T).reshape(E, NT, P)
        # perm order: position p holds global tile p^h
        pos = np.arange(NT) ^ h
        xt_perm = np.ascontiguousarray(xt3[:, pos, :].reshape(E, S)).astype(
            np.float16
        )
        mask = np.concatenate(
            [tri, np.full((P, P), 0.0 if h == 0 else NEG, np.float16)], axis=1
        )
        in_maps.append(
            {
                "xt": xt_perm,
                "wq": np.asarray(Wq, np.float16),
                "wk": np.asarray(Wk, np.float16),
                "wv": np.asarray(Wv, np.float16),
                "bq": np.asarray(bq, np.float32) * np.float32(SCALE),
                "mask": np.ascontiguousarray(mask),
                "ones": np.ones((P, 1), dtype=np.float16),
            }
        )
        # storage col -> global q row (position tile p holds global tile p^h)
        perm = np.empty(S, dtype=np.int64)
        for t in range(NT):
            perm[t * P : (t + 1) * P] = (t ^ h) * P + np.arange(P)
        perms.append(perm)
    return in_maps, perms


def kernel(x, Wq, bq, Wk, bk, Wv, bv):
    from concourse.bass_utils import run_bass_kernel_spmd

    nc = _get_module()
    in_maps, perms = _host_prep(x, Wq, bq, Wk, bk, Wv, bv)
    res = run_bass_kernel_spmd(
        nc,
        in_maps,
        core_ids=list(range(8)),
        trace=TRACE,
        **TRACE_KW,
    )
    _CACHE["last_result"] = res

    bv64 = np.asarray(bv, np.float64)
    out = np.empty((B, S, D), dtype=np.float32)
    for b in range(B):
        r0, r1 = res.results[2 * b], res.results[2 * b + 1]
        pv = np.zeros((D, S), dtype=np.float64)
        sm = np.zeros((S,), dtype=np.float64)
        for r, perm in ((r0, perms[2 * b]), (r1, perms[2 * b + 1])):
            pv[:, perm] += r["pvt"].astype(np.float64)
            sm[perm] += r["sums"][0].astype(np.float64)
        out[b] = ((pv / sm[None, :]).T + bv64[None, :]).astype(np.float32)
    return out


# revision 22
# speedup vs baseline: 832.3179x; 29.0066x over previous
"""Causal self-attention (B=4, S=2048, E=1024, D=128, single head) on 8 TRN2 cores.

Sharding: core c = 2*b + h handles batch b; the two cores of a pair split the
causal key range by k-tile parity (h=0 even 128-row k-tiles, h=1 odd). All 8
cores run the *same* instruction stream; per-core differences live in DRAM
data only:
  - xt [1024, 2048] fp16: x[b].T with 128-col s-tiles stored in "perm order"
    (position p holds global tile p^h), so EVEN positions are always the
    core's own-parity tiles. K/V projection reads even positions via a
    strided AP; Q projection reads all positions in storage order.
  - mask [128, 256] fp16: additive score mask for the two diagonal-region
    k-tiles of every q-block ([tri | 0] for h=0, [tri | -6e4] for h=1).

Math shortcuts vs the reference:
  - K bias dropped entirely: softmax(q.(k+bk)) == softmax(q.k) since the
    q.bk term is constant across keys for a fixed query row.
  - V bias moved to the host: rows of normalized attention sum to 1, so
    out = (pv/sums) + bv exactly.
  - V is projected directly in NATURAL orientation (stationary = 128x128
    x-block, moving = Wv e-tile), killing the 8 PE transposes (~275ns each
    on HW) and the identity tensor.

Engine split (Pool/gpsimd has no PSUM port): PE matmuls; ACT exp + pv/sums
staging copies; DVE mask adds, exp-accumulation adds, and K/Q/V PSUM->SBUF
copies; Pool drives the output DMA ring. PV output is stored fp16
(pvt [128 d, 2048 q perm]); denominators stay f32.

Each core emits unnormalized PV partials and denominators (sums [1, 2048]);
the host un-permutes, combines the pair, and adds bv:
  out[b] = ((pv0 + pv1) / (s0 + s1)).T + bv

Measured: rel err 5.4e-4 on HW. A/Bs were decided with a corrected
TimelineSim (fp16 matmul at 2 cols/cycle, transposes 275ns): V-natural +
engine resplit sims 46051 -> 44082 ns vs the staged baseline; exp-merging
(paired 1024-wide exps into a two-half accumulator) and PE p-state warmup
spam both simmed WORSE and were rejected.
"""

import os

os.environ.setdefault("MYCRO_LOCAL_CACHE", "1")

import numpy as np

B, S, E, D = 4, 2048, 1024, 128
P = 128
NT = S // P          # 16 global s-tiles per batch
LT = NT // 2         # 8 local (per-core) k-tiles
NQB = 4              # 512-wide query blocks
QBW = 512
NEB = E // P         # 8 e-tiles
SCALE = 1.0 / float(np.sqrt(D))
NEG = -60000.0       # fp16-representable; exp underflows to 0 in fp32

TRACE = False        # set by test.py for profiling runs
TRACE_KW = {}
PROBE_NO_XT = False  # timing probe: skip xt DMA (results wrong; perf only)

_CACHE = {}


def _build_module(reps=1, adds="dve", staging="act", vncopy="dve",
                  sc_bufs=4, v_bufs=1, pp_bufs=4, sum_eng="act",
                  fin_delay=True):
    """adds: engine for exp-accumulation adds ('dve'|'pool'|'mix' = small
    masked-tile adds on Pool, rest DVE). staging: engine for pv/sums
    PSUM->SBUF copies. vncopy: engine for V-natural PSUM->SBUF copies."""
    from contextlib import ExitStack

    import concourse.bacc as bacc
    import concourse.mybir as mybir
    import concourse.tile as tile

    f32 = mybir.dt.float32
    f16 = mybir.dt.float16

    nc = bacc.Bacc("TRN2", target_bir_lowering=False, debug=False, num_devices=8)

    xt_d = nc.dram_tensor("xt", [E, S], f16, kind="ExternalInput").ap()
    wq_d = nc.dram_tensor("wq", [E, D], f16, kind="ExternalInput").ap()
    wk_d = nc.dram_tensor("wk", [E, D], f16, kind="ExternalInput").ap()
    wv_d = nc.dram_tensor("wv", [E, D], f16, kind="ExternalInput").ap()
    bq_d = nc.dram_tensor("bq", [D], f32, kind="ExternalInput").ap()  # pre-scaled
    mask_d = nc.dram_tensor("mask", [P, 2 * P], f16, kind="ExternalInput").ap()
    ones_d = nc.dram_tensor("ones", [P, 1], f16, kind="ExternalInput").ap()
    pvt_d = nc.dram_tensor("pvt", [D, S], f16, kind="ExternalOutput").ap()
    sums_d = nc.dram_tensor("sums", [1, S], f32, kind="ExternalOutput").ap()

    with tile.TileContext(nc) as tc, ExitStack() as ctx:
        singles = ctx.enter_context(tc.tile_pool(name="singles", bufs=1))
        ppool = ctx.enter_context(tc.tile_pool(name="ppool", bufs=pp_bufs))
        apool = ctx.enter_context(tc.tile_pool(name="apool", bufs=2))
        proj_ps = ctx.enter_context(tc.tile_pool(name="proj_ps", bufs=1, space="PSUM"))
        v_ps = ctx.enter_context(tc.tile_pool(name="v_ps", bufs=v_bufs, space="PSUM"))
        sc_ps = ctx.enter_context(tc.tile_pool(name="sc_ps", bufs=sc_bufs, space="PSUM"))
        pv_ps = ctx.enter_context(tc.tile_pool(name="pv_ps", bufs=1, space="PSUM"))
        sum_ps = ctx.enter_context(tc.tile_pool(name="sum_ps", bufs=1, space="PSUM"))

        # ---- constants (ACT HWDGE ring; xt stream owns the SP ring) ----
        w_sb = {}
        for name, dram in (("wk", wk_d), ("wv", wv_d), ("wq", wq_d)):
            t = singles.tile([P, NEB, D], f16, tag=f"w_{name}")
            nc.scalar.dma_start(t[:], dram.rearrange("(o p) d -> p o d", p=P))
            w_sb[name] = t
        bq_sb = singles.tile([P, 1], f32, tag="b_bq")
        nc.scalar.dma_start(bq_sb[:], bq_d.rearrange("(p one) -> p one", one=1))
        mask_sb = singles.tile([P, 2 * P], f16, tag="mask")
        nc.scalar.dma_start(mask_sb[:], mask_d[:])
        ones = singles.tile([P, 1], f16, tag="ones")
        nc.scalar.dma_start(ones[:], ones_d[:])

        # ---- persistent activations ----
        # xt resident in perm order: [e-part, e-tile, pair, parity-pos, col]
        xt_sb = singles.tile([P, NEB, LT, 2, P], f16, tag="xt_sb")
        kt = singles.tile([P, LT, P], f16, tag="kt")      # K^T  [d, lt, k]
        vn = singles.tile([P, LT, D], f16, tag="vn")      # V natural [k, lt, d]
        qt = singles.tile([P, NT, P], f16, tag="qt")      # Q^T [d, pos, q]
        pvt_sb = singles.tile([D, S], f16, tag="pvt_sb")
        sums_sb = singles.tile([1, S], f32, tag="sums_sb")

        ktv = kt.rearrange("p lt k -> p (lt k)")

        def load_chunk(J, rep=0):
            """DMA xt columns [512J, 512J+512) (positions 4J..4J+3)."""
            if PROBE_NO_XT and rep > 0:
                return
            for eo in range(NEB):
                nc.sync.dma_start(
                    xt_sb[:, eo, 2 * J : 2 * J + 2, :, :],
                    xt_d[eo * P : (eo + 1) * P, J * QBW : (J + 1) * QBW],
                )

        def proj_chunk(J):
            """K/V for local k-tiles {2J, 2J+1} + Q for block J."""
            # K^T: stationary Wk e-tile, moving x evens; no bias (see docstring)
            ps = proj_ps.tile([P, QBW], f32, tag="ps_kq")
            for eo in range(NEB):
                nc.tensor.matmul(
                    ps[:, : 2 * P],
                    w_sb["wk"][:, eo, :],
                    xt_sb[:, eo, 2 * J : 2 * J + 2, 0, :],
                    start=(eo == 0),
                    stop=(eo == NEB - 1),
                )
            nc.vector.tensor_copy(
                out=ktv[:, J * 2 * P : (J + 1) * 2 * P], in_=ps[:, : 2 * P]
            )
            # V natural: stationary x-block [e, s-tile], moving Wv e-tile
            for lt in (2 * J, 2 * J + 1):
                vps = v_ps.tile([P, D], f32, tag="ps_v")
                for eo in range(NEB):
                    nc.tensor.matmul(
                        vps[:],
                        xt_sb[:, eo, lt, 0, :],
                        w_sb["wv"][:, eo, :],
                        start=(eo == 0),
                        stop=(eo == NEB - 1),
                    )
                if vncopy == "dve":
                    nc.vector.tensor_copy(out=vn[:, lt, :], in_=vps[:])
                else:
                    nc.scalar.copy(out=vn[:, lt, :], in_=vps[:])
            # Q^T over all 4 positions of the chunk
            ps = proj_ps.tile([P, QBW], f32, tag="ps_kq")
            for eo in range(NEB):
                nc.tensor.matmul(
                    ps[:],
                    w_sb["wq"][:, eo, :],
                    xt_sb[:, eo, 2 * J : 2 * J + 2, :, :],
                    start=(eo == 0),
                    stop=(eo == NEB - 1),
                )
            qv = qt.rearrange("p t q -> p (t q)")
            nc.vector.tensor_scalar(
                qv[:, J * QBW : (J + 1) * QBW],
                ps[:],
                SCALE,
                bq_sb[:],
                mybir.AluOpType.mult,
                mybir.AluOpType.add,
            )

        def attention_blk(J):
            """Block J: q-cols [512J, 512J+512), local k-tiles 0..2J+1."""
            nlt = 2 * J + 2
            pv = pv_ps.tile([P, QBW], f32, tag="pv")
            acc = apool.tile([P, QBW], f16, tag="acc")
            # emission order: full-width i=0 first (clears PSUM over the
            # whole block), then the masked tiles (their DVE-mask + exp
            # latency hides behind the remaining scores matmuls), ending on
            # a plain tile so only one exp latency is exposed at block end.
            if J == 0:
                order = [0, 1]
            else:
                order = [0, 2 * J, 2 * J + 1] + list(range(1, 2 * J))
            for idx, i in enumerate(order):
                c0 = 2 * P if i == 2 * J + 1 else 0
                sc = sc_ps.tile([P, QBW], f32, tag="sc")
                nc.tensor.matmul(
                    sc[:, c0:],
                    kt[:, i, :],
                    qt[:, 4 * J + c0 // P : 4 * J + 4, :],
                    start=True,
                    stop=True,
                )
                if i >= 2 * J:
                    nc.vector.tensor_tensor(
                        out=sc[:, c0 : c0 + 2 * P],
                        in0=sc[:, c0 : c0 + 2 * P],
                        in1=mask_sb[:],
                        op=mybir.AluOpType.add,
                    )
                if idx == 0:
                    psrc = acc
                    nc.scalar.activation(
                        acc[:], sc[:], mybir.ActivationFunctionType.Exp
                    )
                else:
                    psrc = ppool.tile([P, QBW], f16, tag="p")
                    nc.scalar.activation(
                        psrc[:, c0:], sc[:, c0:], mybir.ActivationFunctionType.Exp
                    )
                    if adds == "mix":
                        add_eng = nc.gpsimd if c0 else nc.vector
                    else:
                        add_eng = nc.vector if adds == "dve" else nc.gpsimd
                    add_eng.tensor_tensor(
                        out=acc[:, c0:],
                        in0=acc[:, c0:],
                        in1=psrc[:, c0:],
                        op=mybir.AluOpType.add,
                    )
                nc.tensor.matmul(
                    pv[:, c0:],
                    vn[:, i, :],
                    psrc[:, c0:],
                    start=(idx == 0),
                    stop=(idx == nlt - 1),
                    skip_group_check=True,
                )
            return pv, acc

        def finish_blk(J, pv, acc):
            """Block J tail: denominator matmul, staging copies, output DMA."""
            col0 = J * QBW
            sm = sum_ps.tile([1, QBW], f32, tag="sm")
            nc.tensor.matmul(sm[:], ones[:], acc[:], start=True, stop=True)
            if staging == "act":
                nc.scalar.copy(out=pvt_sb[:, col0 : col0 + QBW], in_=pv[:])
            else:
                nc.vector.tensor_copy(out=pvt_sb[:, col0 : col0 + QBW], in_=pv[:])
            if sum_eng == "act":
                nc.scalar.copy(out=sums_sb[:, col0 : col0 + QBW], in_=sm[:])
            else:
                nc.vector.tensor_copy(out=sums_sb[:, col0 : col0 + QBW], in_=sm[:])
            out_eng = nc.gpsimd
            out_eng.dma_start(
                pvt_d[:, col0 : col0 + QBW], pvt_sb[:, col0 : col0 + QBW]
            )
            out_eng.dma_start(
                sums_d[:, col0 : col0 + QBW], sums_sb[:, col0 : col0 + QBW]
            )

        pend = None
        for _rep in range(reps):
            for J in range(NQB):
                load_chunk(J, _rep)
                proj_chunk(J)
                if not fin_delay:
                    finish_blk(J, *attention_blk(J))
                    continue
                if pend is not None:
                    finish_blk(*pend)
                pend = (J, *attention_blk(J))
        if pend is not None:
            finish_blk(*pend)

    nc.compile()
    return nc


def _get_module(reps=1, **kw):
    key = ("nc", reps, tuple(sorted(kw.items())))
    if key not in _CACHE:
        _CACHE[key] = _build_module(reps, **kw)
    return _CACHE[key]


def _host_prep(x, Wq, bq, Wk, bk, Wv, bv):
    """Build the 8 per-core input maps plus per-core q-column permutations."""
    x = np.asarray(x, dtype=np.float32)
    tri = np.where(
        np.arange(P)[None, :] >= np.arange(P)[:, None], 0.0, NEG
    ).astype(np.float16)
    in_maps = []
    perms = []
    for c in range(8):
        b, h = divmod(c, 2)
        xt3 = np.ascontiguousarray(x[b].T).reshape(E, NT, P)
        # perm order: position p holds global tile p^h
        pos = np.arange(NT) ^ h
        xt_perm = np.ascontiguousarray(xt3[:, pos, :].reshape(E, S)).astype(
            np.float16
        )
        mask = np.concatenate(
            [tri, np.full((P, P), 0.0 if h == 0 else NEG, np.float16)], axis=1
        )
        in_maps.append(
            {
                "xt": xt_perm,
                "wq": np.asarray(Wq, np.float16),
                "wk": np.asarray(Wk, np.float16),
                "wv": np.asarray(Wv, np.float16),
                "bq": np.asarray(bq, np.float32) * np.float32(SCALE),
                "mask": np.ascontiguousarray(mask),
                "ones": np.ones((P, 1), dtype=np.float16),
            }
        )
        # storage col -> global q row (position tile p holds global tile p^h)
        perm = np.empty(S, dtype=np.int64)
        for t in range(NT):
            perm[t * P : (t + 1) * P] = (t ^ h) * P + np.arange(P)
        perms.append(perm)
    return in_maps, perms


def kernel(x, Wq, bq, Wk, bk, Wv, bv):
    from concourse.bass_utils import run_bass_kernel_spmd

    nc = _get_module()
    in_maps, perms = _host_prep(x, Wq, bq, Wk, bk, Wv, bv)
    res = run_bass_kernel_spmd(
        nc,
        in_maps,
        core_ids=list(range(8)),
        trace=TRACE,
        **TRACE_KW,
    )
    _CACHE["last_result"] = res

    bv64 = np.asarray(bv, np.float64)
    out = np.empty((B, S, D), dtype=np.float32)
    for b in range(B):
        r0, r1 = res.results[2 * b], res.results[2 * b + 1]
        pv = np.zeros((D, S), dtype=np.float64)
        sm = np.zeros((S,), dtype=np.float64)
        for r, perm in ((r0, perms[2 * b]), (r1, perms[2 * b + 1])):
            pv[:, perm] += r["pvt"].astype(np.float64)
            sm[perm] += r["sums"][0].astype(np.float64)
        out[b] = ((pv / sm[None, :]).T + bv64[None, :]).astype(np.float32)
    return out


# revision 24
# speedup vs baseline: 851.2156x; 1.0227x over previous
"""Causal self-attention (B=4, S=2048, E=1024, D=128, single head) on 8 TRN2 cores.

Sharding: core c = 2*b + h handles batch b; the two cores of a pair split the
causal key range by k-tile parity (h=0 even 128-row k-tiles, h=1 odd). All 8
cores run the *same* instruction stream; per-core differences live in DRAM
data only:
  - xt [1024, 2048] fp16: x[b].T with 128-col s-tiles stored in "perm order"
    (position p holds global tile p^h), so EVEN positions are always the
    core's own-parity tiles. K/V projection reads even positions via a
    strided AP; Q projection reads all positions in storage order.
  - mask [128, 256] fp16: additive score mask for the two diagonal-region
    k-tiles of every q-block ([tri | 0] for h=0, [tri | -6e4] for h=1).

Math shortcuts vs the reference:
  - K bias dropped entirely: softmax(q.(k+bk)) == softmax(q.k) since the
    q.bk term is constant across keys for a fixed query row.
  - V bias moved to the host: rows of normalized attention sum to 1, so
    out = (pv/sums) + bv exactly.
  - V is projected directly in NATURAL orientation (stationary = 128x128
    x-block, moving = Wv e-tile), killing the 8 PE transposes (~275ns each
    on HW) and the identity tensor.

Engine split (Pool/gpsimd has no PSUM port): PE matmuls; ACT exp + pv/sums
staging copies; DVE mask adds, exp-accumulation adds, and K/Q/V PSUM->SBUF
copies; Pool drives the output DMA ring. PV output is stored fp16
(pvt [128 d, 2048 q perm]); denominators stay f32.

Each core emits unnormalized PV partials and denominators (sums [1, 2048]);
the host un-permutes, combines the pair, and adds bv:
  out[b] = ((pv0 + pv1) / (s0 + s1)).T + bv

Measured: rel err 5.4e-4 on HW. A/Bs were decided with a corrected
TimelineSim (fp16 matmul at 2 cols/cycle, transposes 275ns; simcmp.py):
the staged baseline sims at 46051 ns, this kernel at 42590 ns (-7.5%), via
V-natural projection + engine resplit (44082), separate K/Q psum banks so
Q-proj matmuls never wait on the K psum->SBUF copy (proj_split, -967ns),
and emitting each block's denominator/staging after the NEXT chunk's
projections so the PE never stalls on the DVE accumulation tail
(fin_delay, -208ns). Rejected by the same estimator: exp-merging (paired
1024-wide exps into a two-half accumulator), PE p-state warmup spam,
Pool-engine accumulation adds, and masked-tiles-last emission. fp8
(DoubleRow) Q/K projection was rejected for accuracy (rel err 5.5e-2 on
HW vs the 2e-2 gate).
"""

import os

os.environ.setdefault("MYCRO_LOCAL_CACHE", "1")

import numpy as np

B, S, E, D = 4, 2048, 1024, 128
P = 128
NT = S // P          # 16 global s-tiles per batch
LT = NT // 2         # 8 local (per-core) k-tiles
NQB = 4              # 512-wide query blocks
QBW = 512
NEB = E // P         # 8 e-tiles
SCALE = 1.0 / float(np.sqrt(D))
NEG = -60000.0       # fp16-representable; exp underflows to 0 in fp32

TRACE = False        # set by test.py for profiling runs
TRACE_KW = {}
PROBE_NO_XT = False  # timing probe: skip xt DMA (results wrong; perf only)

_CACHE = {}


def _build_module(reps=1, adds="dve", staging="act", vncopy="dve",
                  sc_bufs=3, v_bufs=1, pp_bufs=4, sum_eng="act",
                  fin_delay=True, masked_last=False, proj_split=True):
    """adds: engine for exp-accumulation adds ('dve'|'pool'|'mix' = small
    masked-tile adds on Pool, rest DVE). staging: engine for pv/sums
    PSUM->SBUF copies. vncopy: engine for V-natural PSUM->SBUF copies."""
    from contextlib import ExitStack

    import concourse.bacc as bacc
    import concourse.mybir as mybir
    import concourse.tile as tile

    f32 = mybir.dt.float32
    f16 = mybir.dt.float16

    nc = bacc.Bacc("TRN2", target_bir_lowering=False, debug=False, num_devices=8)

    xt_d = nc.dram_tensor("xt", [E, S], f16, kind="ExternalInput").ap()
    wq_d = nc.dram_tensor("wq", [E, D], f16, kind="ExternalInput").ap()
    wk_d = nc.dram_tensor("wk", [E, D], f16, kind="ExternalInput").ap()
    wv_d = nc.dram_tensor("wv", [E, D], f16, kind="ExternalInput").ap()
    bq_d = nc.dram_tensor("bq", [D], f32, kind="ExternalInput").ap()  # pre-scaled
    mask_d = nc.dram_tensor("mask", [P, 2 * P], f16, kind="ExternalInput").ap()
    ones_d = nc.dram_tensor("ones", [P, 1], f16, kind="ExternalInput").ap()
    pvt_d = nc.dram_tensor("pvt", [D, S], f16, kind="ExternalOutput").ap()
    sums_d = nc.dram_tensor("sums", [1, S], f32, kind="ExternalOutput").ap()

    with tile.TileContext(nc) as tc, ExitStack() as ctx:
        singles = ctx.enter_context(tc.tile_pool(name="singles", bufs=1))
        ppool = ctx.enter_context(tc.tile_pool(name="ppool", bufs=pp_bufs))
        apool = ctx.enter_context(tc.tile_pool(name="apool", bufs=2))
        proj_ps = ctx.enter_context(tc.tile_pool(name="proj_ps", bufs=1, space="PSUM"))
        v_ps = ctx.enter_context(tc.tile_pool(name="v_ps", bufs=v_bufs, space="PSUM"))
        sc_ps = ctx.enter_context(tc.tile_pool(name="sc_ps", bufs=sc_bufs, space="PSUM"))
        pv_ps = ctx.enter_context(tc.tile_pool(name="pv_ps", bufs=1, space="PSUM"))
        sum_ps = ctx.enter_context(tc.tile_pool(name="sum_ps", bufs=1, space="PSUM"))

        # ---- constants (ACT HWDGE ring; xt stream owns the SP ring) ----
        w_sb = {}
        for name, dram in (("wk", wk_d), ("wv", wv_d), ("wq", wq_d)):
            t = singles.tile([P, NEB, D], f16, tag=f"w_{name}")
            nc.scalar.dma_start(t[:], dram.rearrange("(o p) d -> p o d", p=P))
            w_sb[name] = t
        bq_sb = singles.tile([P, 1], f32, tag="b_bq")
        nc.scalar.dma_start(bq_sb[:], bq_d.rearrange("(p one) -> p one", one=1))
        mask_sb = singles.tile([P, 2 * P], f16, tag="mask")
        nc.scalar.dma_start(mask_sb[:], mask_d[:])
        ones = singles.tile([P, 1], f16, tag="ones")
        nc.scalar.dma_start(ones[:], ones_d[:])

        # ---- persistent activations ----
        # xt resident in perm order: [e-part, e-tile, pair, parity-pos, col]
        xt_sb = singles.tile([P, NEB, LT, 2, P], f16, tag="xt_sb")
        kt = singles.tile([P, LT, P], f16, tag="kt")      # K^T  [d, lt, k]
        vn = singles.tile([P, LT, D], f16, tag="vn")      # V natural [k, lt, d]
        qt = singles.tile([P, NT, P], f16, tag="qt")      # Q^T [d, pos, q]
        pvt_sb = singles.tile([D, S], f16, tag="pvt_sb")
        sums_sb = singles.tile([1, S], f32, tag="sums_sb")

        ktv = kt.rearrange("p lt k -> p (lt k)")

        def load_chunk(J, rep=0):
            """DMA xt columns [512J, 512J+512) (positions 4J..4J+3)."""
            if PROBE_NO_XT and rep > 0:
                return
            for eo in range(NEB):
                nc.sync.dma_start(
                    xt_sb[:, eo, 2 * J : 2 * J + 2, :, :],
                    xt_d[eo * P : (eo + 1) * P, J * QBW : (J + 1) * QBW],
                )

        def proj_chunk(J):
            """K/V for local k-tiles {2J, 2J+1} + Q for block J."""
            # K^T: stationary Wk e-tile, moving x evens; no bias (see docstring)
            ps = proj_ps.tile([P, QBW], f32, tag="ps_k" if proj_split else "ps_kq")
            for eo in range(NEB):
                nc.tensor.matmul(
                    ps[:, : 2 * P],
                    w_sb["wk"][:, eo, :],
                    xt_sb[:, eo, 2 * J : 2 * J + 2, 0, :],
                    start=(eo == 0),
                    stop=(eo == NEB - 1),
                )
            nc.vector.tensor_copy(
                out=ktv[:, J * 2 * P : (J + 1) * 2 * P], in_=ps[:, : 2 * P]
            )
            # V natural: stationary x-block [e, s-tile], moving Wv e-tile
            for lt in (2 * J, 2 * J + 1):
                vps = v_ps.tile([P, D], f32, tag="ps_v")
                for eo in range(NEB):
                    nc.tensor.matmul(
                        vps[:],
                        xt_sb[:, eo, lt, 0, :],
                        w_sb["wv"][:, eo, :],
                        start=(eo == 0),
                        stop=(eo == NEB - 1),
                    )
                if vncopy == "dve":
                    nc.vector.tensor_copy(out=vn[:, lt, :], in_=vps[:])
                else:
                    nc.scalar.copy(out=vn[:, lt, :], in_=vps[:])
            # Q^T over all 4 positions of the chunk
            ps = proj_ps.tile([P, QBW], f32, tag="ps_q" if proj_split else "ps_kq")
            for eo in range(NEB):
                nc.tensor.matmul(
                    ps[:],
                    w_sb["wq"][:, eo, :],
                    xt_sb[:, eo, 2 * J : 2 * J + 2, :, :],
                    start=(eo == 0),
                    stop=(eo == NEB - 1),
                )
            qv = qt.rearrange("p t q -> p (t q)")
            nc.vector.tensor_scalar(
                qv[:, J * QBW : (J + 1) * QBW],
                ps[:],
                SCALE,
                bq_sb[:],
                mybir.AluOpType.mult,
                mybir.AluOpType.add,
            )

        def attention_blk(J):
            """Block J: q-cols [512J, 512J+512), local k-tiles 0..2J+1."""
            nlt = 2 * J + 2
            pv = pv_ps.tile([P, QBW], f32, tag="pv")
            acc = apool.tile([P, QBW], f16, tag="acc")
            # emission order: full-width i=0 first (clears PSUM over the
            # whole block), then the masked tiles (their DVE-mask + exp
            # latency hides behind the remaining scores matmuls), ending on
            # a plain tile so only one exp latency is exposed at block end.
            if J == 0:
                order = [0, 1]
            elif masked_last:
                order = [0] + list(range(1, 2 * J)) + [2 * J, 2 * J + 1]
            else:
                order = [0, 2 * J, 2 * J + 1] + list(range(1, 2 * J))
            for idx, i in enumerate(order):
                c0 = 2 * P if i == 2 * J + 1 else 0
                sc = sc_ps.tile([P, QBW], f32, tag="sc")
                nc.tensor.matmul(
                    sc[:, c0:],
                    kt[:, i, :],
                    qt[:, 4 * J + c0 // P : 4 * J + 4, :],
                    start=True,
                    stop=True,
                )
                if i >= 2 * J:
                    nc.vector.tensor_tensor(
                        out=sc[:, c0 : c0 + 2 * P],
                        in0=sc[:, c0 : c0 + 2 * P],
                        in1=mask_sb[:],
                        op=mybir.AluOpType.add,
                    )
                if idx == 0:
                    psrc = acc
                    nc.scalar.activation(
                        acc[:], sc[:], mybir.ActivationFunctionType.Exp
                    )
                else:
                    psrc = ppool.tile([P, QBW], f16, tag="p")
                    nc.scalar.activation(
                        psrc[:, c0:], sc[:, c0:], mybir.ActivationFunctionType.Exp
                    )
                    if adds == "mix":
                        add_eng = nc.gpsimd if c0 else nc.vector
                    else:
                        add_eng = nc.vector if adds == "dve" else nc.gpsimd
                    add_eng.tensor_tensor(
                        out=acc[:, c0:],
                        in0=acc[:, c0:],
                        in1=psrc[:, c0:],
                        op=mybir.AluOpType.add,
                    )
                nc.tensor.matmul(
                    pv[:, c0:],
                    vn[:, i, :],
                    psrc[:, c0:],
                    start=(idx == 0),
                    stop=(idx == nlt - 1),
                    skip_group_check=True,
                )
            return pv, acc

        def finish_blk(J, pv, acc):
            """Block J tail: denominator matmul, staging copies, output DMA."""
            col0 = J * QBW
            sm = sum_ps.tile([1, QBW], f32, tag="sm")
            nc.tensor.matmul(sm[:], ones[:], acc[:], start=True, stop=True)
            if staging == "act":
                nc.scalar.copy(out=pvt_sb[:, col0 : col0 + QBW], in_=pv[:])
            else:
                nc.vector.tensor_copy(out=pvt_sb[:, col0 : col0 + QBW], in_=pv[:])
            if sum_eng == "act":
                nc.scalar.copy(out=sums_sb[:, col0 : col0 + QBW], in_=sm[:])
            else:
                nc.vector.tensor_copy(out=sums_sb[:, col0 : col0 + QBW], in_=sm[:])
            out_eng = nc.gpsimd
            out_eng.dma_start(
                pvt_d[:, col0 : col0 + QBW], pvt_sb[:, col0 : col0 + QBW]
            )
            out_eng.dma_start(
                sums_d[:, col0 : col0 + QBW], sums_sb[:, col0 : col0 + QBW]
            )

        pend = None
        for _rep in range(reps):
            for J in range(NQB):
                load_chunk(J, _rep)
                proj_chunk(J)
                if not fin_delay:
                    finish_blk(J, *attention_blk(J))
                    continue
                if pend is not None:
                    finish_blk(*pend)
                pend = (J, *attention_blk(J))
        if pend is not None:
            finish_blk(*pend)

    nc.compile()
    return nc


def _get_module(reps=1, **kw):
    key = ("nc", reps, tuple(sorted(kw.items())))
    if key not in _CACHE:
        _CACHE[key] = _build_module(reps, **kw)
    return _CACHE[key]


def _host_prep(x, Wq, bq, Wk, bk, Wv, bv):
    """Build the 8 per-core input maps plus per-core q-column permutations."""
    x = np.asarray(x, dtype=np.float32)
    tri = np.where(
        np.arange(P)[None, :] >= np.arange(P)[:, None], 0.0, NEG
    ).astype(np.float16)
    in_maps = []
    perms = []
    for c in range(8):
        b, h = divmod(c, 2)
        xt3 = np.ascontiguousarray(x[b].T).reshape(E, NT, P)
        # perm order: position p holds global tile p^h
        pos = np.arange(NT) ^ h
        xt_perm = np.ascontiguousarray(xt3[:, pos, :].reshape(E, S)).astype(
            np.float16
        )
        mask = np.concatenate(
            [tri, np.full((P, P), 0.0 if h == 0 else NEG, np.float16)], axis=1
        )
        in_maps.append(
            {
                "xt": xt_perm,
                "wq": np.asarray(Wq, np.float16),
                "wk": np.asarray(Wk, np.float16),
                "wv": np.asarray(Wv, np.float16),
                "bq": np.asarray(bq, np.float32) * np.float32(SCALE),
                "mask": np.ascontiguousarray(mask),
                "ones": np.ones((P, 1), dtype=np.float16),
            }
        )
        # storage col -> global q row (position tile p holds global tile p^h)
        perm = np.empty(S, dtype=np.int64)
        for t in range(NT):
            perm[t * P : (t + 1) * P] = (t ^ h) * P + np.arange(P)
        perms.append(perm)
    return in_maps, perms


def kernel(x, Wq, bq, Wk, bk, Wv, bv):
    from concourse.bass_utils import run_bass_kernel_spmd

    nc = _get_module()
    in_maps, perms = _host_prep(x, Wq, bq, Wk, bk, Wv, bv)
    res = run_bass_kernel_spmd(
        nc,
        in_maps,
        core_ids=list(range(8)),
        trace=TRACE,
        **TRACE_KW,
    )
    _CACHE["last_result"] = res

    bv64 = np.asarray(bv, np.float64)
    out = np.empty((B, S, D), dtype=np.float32)
    for b in range(B):
        r0, r1 = res.results[2 * b], res.results[2 * b + 1]
        pv = np.zeros((D, S), dtype=np.float64)
        sm = np.zeros((S,), dtype=np.float64)
        for r, perm in ((r0, perms[2 * b]), (r1, perms[2 * b + 1])):
            pv[:, perm] += r["pvt"].astype(np.float64)
            sm[perm] += r["sums"][0].astype(np.float64)
        out[b] = ((pv / sm[None, :]).T + bv64[None, :]).astype(np.float32)
    return out


# revision 30
# speedup vs baseline: 1128.6119x; 1.3259x over previous
"""Causal self-attention (B=4, S=2048, E=1024, D=128, single head) on 8 TRN2 cores.

Sharding: core c = 2*b + h handles batch b; the two cores of a pair split the
causal key range by k-tile parity (h=0 even 128-row k-tiles, h=1 odd). All 8
cores run the *same* instruction stream; per-core differences live in DRAM
data only:
  - xt [1024, 2048] fp16: x[b].T with 128-col s-tiles stored in "perm order"
    (position p holds global tile p^h), so EVEN positions are always the
    core's own-parity tiles. K/V projection reads even positions via a
    strided AP; Q projection reads all positions in storage order.
  - mask [128, 256] fp16: additive score mask for the two diagonal-region
    k-tiles of every q-block ([tri | 0] for h=0, [tri | -6e4] for h=1).

Math shortcuts vs the reference:
  - K bias dropped entirely: softmax(q.(k+bk)) == softmax(q.k) since the
    q.bk term is constant across keys for a fixed query row.
  - V bias moved to the host: rows of normalized attention sum to 1, so
    out = (pv/sums) + bv exactly.
  - V is projected directly in NATURAL orientation (stationary = 128x128
    x-block, moving = Wv e-tile), killing the 8 PE transposes (~275ns each
    on HW) and the identity tensor.

Engine split (Pool/gpsimd has no PSUM port): PE matmuls; ACT exp + pv/sums
staging copies; DVE mask adds, exp-accumulation adds, and K/Q/V PSUM->SBUF
copies; Pool drives the output DMA ring. PV output is stored fp16
(pvt [128 d, 2048 q perm]); denominators stay f32.

Each core emits unnormalized PV partials and denominators (sums [1, 2048]);
the host un-permutes, combines the pair, and adds bv:
  out[b] = ((pv0 + pv1) / (s0 + s1)).T + bv

Measured: rel err 5.4e-4 on HW. A/Bs were decided with a corrected
TimelineSim (simcmp.py: fp16 matmul at 2 cols/cycle, transposes 275 ns,
DMA rescaled to the ~358 GB/s HBM line rate): the staged baseline sims at
35728 ns, this kernel at 32122 ns (-10%), via
  - V-natural projection + engine resplit (kills 8 PE transposes),
  - separate K/Q psum banks so Q-proj matmuls never wait on the K
    psum->SBUF copy (proj_split),
  - each block's denominator/staging emitted after the NEXT chunk's
    projections so the PE never stalls on the DVE accumulation tail
    (fin_delay),
  - xt chunk DMAs coalesced 8 -> 2 per chunk via a rearranged DRAM AP
    (dma_merge; ~26 fewer SP-queue instructions + semaphores), with chunk
    0 kept at 2-e-tile granularity (dma_merge_j0=4, ~256 KB first group
    ~= the wk weight-DMA arrival time) so the first projection still
    starts early.
Rejected by the same estimator: exp-merging (paired 1024-wide exps into a
two-half accumulator), PE p-state warmup spam, Pool-engine accumulation
adds, masked-tiles-last emission, attention-one-chunk-behind pipelining,
and single-8-e-tile chunk DMAs. fp8 (DoubleRow) Q/K projection was
rejected for accuracy (rel err 5.5e-2 on HW vs the 2e-2 gate).
"""

import os

os.environ.setdefault("MYCRO_LOCAL_CACHE", "1")

import numpy as np

B, S, E, D = 4, 2048, 1024, 128
P = 128
NT = S // P          # 16 global s-tiles per batch
LT = NT // 2         # 8 local (per-core) k-tiles
NQB = 4              # 512-wide query blocks
QBW = 512
NEB = E // P         # 8 e-tiles
SCALE = 1.0 / float(np.sqrt(D))
NEG = -60000.0       # fp16-representable; exp underflows to 0 in fp32

TRACE = False        # set by test.py for profiling runs
TRACE_KW = {}
PROBE_NO_XT = False  # timing probe: skip xt DMA (results wrong; perf only)

_CACHE = {}


def _build_module(reps=1, adds="dve", staging="act", vncopy="dve",
                  sc_bufs=3, v_bufs=1, pp_bufs=4, sum_eng="act",
                  fin_delay=True, masked_last=False, proj_split=True,
                  order_variant=0, dma_merge=2, dma_merge_j0=4,
                  attn_delay=False, ap_bufs=2):
    """adds: engine for exp-accumulation adds ('dve'|'pool'|'mix' = small
    masked-tile adds on Pool, rest DVE). staging: engine for pv/sums
    PSUM->SBUF copies. vncopy: engine for V-natural PSUM->SBUF copies."""
    from contextlib import ExitStack

    import concourse.bacc as bacc
    import concourse.mybir as mybir
    import concourse.tile as tile

    f32 = mybir.dt.float32
    f16 = mybir.dt.float16

    nc = bacc.Bacc("TRN2", target_bir_lowering=False, debug=False, num_devices=8)

    xt_d = nc.dram_tensor("xt", [E, S], f16, kind="ExternalInput").ap()
    wq_d = nc.dram_tensor("wq", [E, D], f16, kind="ExternalInput").ap()
    wk_d = nc.dram_tensor("wk", [E, D], f16, kind="ExternalInput").ap()
    wv_d = nc.dram_tensor("wv", [E, D], f16, kind="ExternalInput").ap()
    bq_d = nc.dram_tensor("bq", [D], f32, kind="ExternalInput").ap()  # pre-scaled
    mask_d = nc.dram_tensor("mask", [P, 2 * P], f16, kind="ExternalInput").ap()
    ones_d = nc.dram_tensor("ones", [P, 1], f16, kind="ExternalInput").ap()
    pvt_d = nc.dram_tensor("pvt", [D, S], f16, kind="ExternalOutput").ap()
    sums_d = nc.dram_tensor("sums", [1, S], f32, kind="ExternalOutput").ap()

    with tile.TileContext(nc) as tc, ExitStack() as ctx:
        singles = ctx.enter_context(tc.tile_pool(name="singles", bufs=1))
        ppool = ctx.enter_context(tc.tile_pool(name="ppool", bufs=pp_bufs))
        apool = ctx.enter_context(tc.tile_pool(name="apool", bufs=ap_bufs))
        proj_ps = ctx.enter_context(tc.tile_pool(name="proj_ps", bufs=1, space="PSUM"))
        v_ps = ctx.enter_context(tc.tile_pool(name="v_ps", bufs=v_bufs, space="PSUM"))
        sc_ps = ctx.enter_context(tc.tile_pool(name="sc_ps", bufs=sc_bufs, space="PSUM"))
        pv_ps = ctx.enter_context(tc.tile_pool(name="pv_ps", bufs=1, space="PSUM"))
        sum_ps = ctx.enter_context(tc.tile_pool(name="sum_ps", bufs=1, space="PSUM"))

        # ---- constants (ACT HWDGE ring; xt stream owns the SP ring) ----
        w_sb = {}
        for name, dram in (("wk", wk_d), ("wv", wv_d), ("wq", wq_d)):
            t = singles.tile([P, NEB, D], f16, tag=f"w_{name}")
            nc.scalar.dma_start(t[:], dram.rearrange("(o p) d -> p o d", p=P))
            w_sb[name] = t
        bq_sb = singles.tile([P, 1], f32, tag="b_bq")
        nc.scalar.dma_start(bq_sb[:], bq_d.rearrange("(p one) -> p one", one=1))
        mask_sb = singles.tile([P, 2 * P], f16, tag="mask")
        nc.scalar.dma_start(mask_sb[:], mask_d[:])
        ones = singles.tile([P, 1], f16, tag="ones")
        nc.scalar.dma_start(ones[:], ones_d[:])

        # ---- persistent activations ----
        # xt resident in perm order: [e-part, e-tile, pair, parity-pos, col]
        xt_sb = singles.tile([P, NEB, LT, 2, P], f16, tag="xt_sb")
        kt = singles.tile([P, LT, P], f16, tag="kt")      # K^T  [d, lt, k]
        vn = singles.tile([P, LT, D], f16, tag="vn")      # V natural [k, lt, d]
        qt = singles.tile([P, NT, P], f16, tag="qt")      # Q^T [d, pos, q]
        pvt_sb = singles.tile([D, S], f16, tag="pvt_sb")
        sums_sb = singles.tile([1, S], f32, tag="sums_sb")

        ktv = kt.rearrange("p lt k -> p (lt k)")

        xt_d3 = xt_d.rearrange("(o p) s -> p o s", p=P)

        def load_chunk(J, rep=0):
            """DMA xt columns [512J, 512J+512) (positions 4J..4J+3).

            nd = DMAs per chunk (1, 2, 4, or 8): fewer, bigger transfers cut
            SP-queue instruction + semaphore overhead; chunk 0 can stay
            fine-grained (dma_merge_j0) so the first projection starts after
            one e-tile instead of the whole 1 MB chunk."""
            if PROBE_NO_XT and rep > 0:
                return
            nd = dma_merge_j0 if (J == 0 and rep == 0) else dma_merge
            step = NEB // nd
            for g in range(nd):
                eo0 = g * step
                nc.sync.dma_start(
                    xt_sb[:, eo0 : eo0 + step, 2 * J : 2 * J + 2, :, :],
                    xt_d3[:, eo0 : eo0 + step, J * QBW : (J + 1) * QBW],
                )

        def proj_chunk(J):
            """K/V for local k-tiles {2J, 2J+1} + Q for block J."""
            # K^T: stationary Wk e-tile, moving x evens; no bias (see docstring)
            ps = proj_ps.tile([P, QBW], f32, tag="ps_k" if proj_split else "ps_kq")
            for eo in range(NEB):
                nc.tensor.matmul(
                    ps[:, : 2 * P],
                    w_sb["wk"][:, eo, :],
                    xt_sb[:, eo, 2 * J : 2 * J + 2, 0, :],
                    start=(eo == 0),
                    stop=(eo == NEB - 1),
                )
            nc.vector.tensor_copy(
                out=ktv[:, J * 2 * P : (J + 1) * 2 * P], in_=ps[:, : 2 * P]
            )
            # V natural: stationary x-block [e, s-tile], moving Wv e-tile
            for lt in (2 * J, 2 * J + 1):
                vps = v_ps.tile([P, D], f32, tag="ps_v")
                for eo in range(NEB):
                    nc.tensor.matmul(
                        vps[:],
                        xt_sb[:, eo, lt, 0, :],
                        w_sb["wv"][:, eo, :],
                        start=(eo == 0),
                        stop=(eo == NEB - 1),
                    )
                if vncopy == "dve":
                    nc.vector.tensor_copy(out=vn[:, lt, :], in_=vps[:])
                else:
                    nc.scalar.copy(out=vn[:, lt, :], in_=vps[:])
            # Q^T over all 4 positions of the chunk
            ps = proj_ps.tile([P, QBW], f32, tag="ps_q" if proj_split else "ps_kq")
            for eo in range(NEB):
                nc.tensor.matmul(
                    ps[:],
                    w_sb["wq"][:, eo, :],
                    xt_sb[:, eo, 2 * J : 2 * J + 2, :, :],
                    start=(eo == 0),
                    stop=(eo == NEB - 1),
                )
            qv = qt.rearrange("p t q -> p (t q)")
            nc.vector.tensor_scalar(
                qv[:, J * QBW : (J + 1) * QBW],
                ps[:],
                SCALE,
                bq_sb[:],
                mybir.AluOpType.mult,
                mybir.AluOpType.add,
            )

        def attention_blk(J):
            """Block J: q-cols [512J, 512J+512), local k-tiles 0..2J+1."""
            nlt = 2 * J + 2
            pv = pv_ps.tile([P, QBW], f32, tag="pv")
            acc = apool.tile([P, QBW], f16, tag="acc")
            # emission order: full-width i=0 first (clears PSUM over the
            # whole block), then the masked tiles (their DVE-mask + exp
            # latency hides behind the remaining scores matmuls), ending on
            # a plain tile so only one exp latency is exposed at block end.
            if J == 0:
                order = [0, 1]
            elif masked_last:
                order = [0] + list(range(1, 2 * J)) + [2 * J, 2 * J + 1]
            elif order_variant == 1 and J >= 1:
                order = [0, 2 * J, 1, 2 * J + 1] + list(range(2, 2 * J))
                order = [t for t in order if t < 2 * J + 2][:2 * J + 2]
                if J == 1:
                    order = [0, 2, 3]
            elif order_variant == 2 and J >= 1:
                order = [2 * J, 0, 2 * J + 1] + list(range(1, 2 * J))
            else:
                order = [0, 2 * J, 2 * J + 1] + list(range(1, 2 * J))
            for idx, i in enumerate(order):
                c0 = 2 * P if i == 2 * J + 1 else 0
                sc = sc_ps.tile([P, QBW], f32, tag="sc")
                nc.tensor.matmul(
                    sc[:, c0:],
                    kt[:, i, :],
                    qt[:, 4 * J + c0 // P : 4 * J + 4, :],
                    start=True,
                    stop=True,
                )
                if i >= 2 * J:
                    nc.vector.tensor_tensor(
                        out=sc[:, c0 : c0 + 2 * P],
                        in0=sc[:, c0 : c0 + 2 * P],
                        in1=mask_sb[:],
                        op=mybir.AluOpType.add,
                    )
                if idx == 0:
                    psrc = acc
                    nc.scalar.activation(
                        acc[:], sc[:], mybir.ActivationFunctionType.Exp
                    )
                else:
                    psrc = ppool.tile([P, QBW], f16, tag="p")
                    nc.scalar.activation(
                        psrc[:, c0:], sc[:, c0:], mybir.ActivationFunctionType.Exp
                    )
                    if adds == "mix":
                        add_eng = nc.gpsimd if c0 else nc.vector
                    else:
                        add_eng = nc.vector if adds == "dve" else nc.gpsimd
                    add_eng.tensor_tensor(
                        out=acc[:, c0:],
                        in0=acc[:, c0:],
                        in1=psrc[:, c0:],
                        op=mybir.AluOpType.add,
                    )
                nc.tensor.matmul(
                    pv[:, c0:],
                    vn[:, i, :],
                    psrc[:, c0:],
                    start=(idx == 0),
                    stop=(idx == nlt - 1),
                    skip_group_check=True,
                )
            return pv, acc

        def finish_blk(J, pv, acc):
            """Block J tail: denominator matmul, staging copies, output DMA."""
            col0 = J * QBW
            sm = sum_ps.tile([1, QBW], f32, tag="sm")
            nc.tensor.matmul(sm[:], ones[:], acc[:], start=True, stop=True)
            if staging == "act":
                nc.scalar.copy(out=pvt_sb[:, col0 : col0 + QBW], in_=pv[:])
            else:
                nc.vector.tensor_copy(out=pvt_sb[:, col0 : col0 + QBW], in_=pv[:])
            if sum_eng == "act":
                nc.scalar.copy(out=sums_sb[:, col0 : col0 + QBW], in_=sm[:])
            else:
                nc.vector.tensor_copy(out=sums_sb[:, col0 : col0 + QBW], in_=sm[:])
            out_eng = nc.gpsimd
            out_eng.dma_start(
                pvt_d[:, col0 : col0 + QBW], pvt_sb[:, col0 : col0 + QBW]
            )
            out_eng.dma_start(
                sums_d[:, col0 : col0 + QBW], sums_sb[:, col0 : col0 + QBW]
            )

        pend = None
        pend_attn = None
        for _rep in range(reps):
            for J in range(NQB):
                load_chunk(J, _rep)
                proj_chunk(J)
                if attn_delay:
                    if pend is not None:
                        finish_blk(*pend)
                        pend = None
                    if pend_attn is not None:
                        pend = (pend_attn, *attention_blk(pend_attn))
                    pend_attn = J
                    continue
                if not fin_delay:
                    finish_blk(J, *attention_blk(J))
                    continue
                if pend is not None:
                    finish_blk(*pend)
                pend = (J, *attention_blk(J))
        if pend is not None:
            finish_blk(*pend)
            pend = None
        if pend_attn is not None:
            finish_blk(pend_attn, *attention_blk(pend_attn))

    nc.compile()
    return nc


def _get_module(reps=1, **kw):
    key = ("nc", reps, tuple(sorted(kw.items())))
    if key not in _CACHE:
        _CACHE[key] = _build_module(reps, **kw)
    return _CACHE[key]


def _host_prep(x, Wq, bq, Wk, bk, Wv, bv):
    """Build the 8 per-core input maps plus per-core q-column permutations."""
    x = np.asarray(x, dtype=np.float32)
    tri = np.where(
        np.arange(P)[None, :] >= np.arange(P)[:, None], 0.0, NEG
    ).astype(np.float16)
    in_maps = []
    perms = []
    for c in range(8):
        b, h = divmod(c, 2)
        xt3 = np.ascontiguousarray(x[b].T).reshape(E, NT, P)
        # perm order: position p holds global tile p^h
        pos = np.arange(NT) ^ h
        xt_perm = np.ascontiguousarray(xt3[:, pos, :].reshape(E, S)).astype(
            np.float16
        )
        mask = np.concatenate(
            [tri, np.full((P, P), 0.0 if h == 0 else NEG, np.float16)], axis=1
        )
        in_maps.append(
            {
                "xt": xt_perm,
                "wq": np.asarray(Wq, np.float16),
                "wk": np.asarray(Wk, np.float16),
                "wv": np.asarray(Wv, np.float16),
                "bq": np.asarray(bq, np.float32) * np.float32(SCALE),
                "mask": np.ascontiguousarray(mask),
                "ones": np.ones((P, 1), dtype=np.float16),
            }
        )
        # storage col -> global q row (position tile p holds global tile p^h)
        perm = np.empty(S, dtype=np.int64)
        for t in range(NT):
            perm[t * P : (t + 1) * P] = (t ^ h) * P + np.arange(P)
        perms.append(perm)
    return in_maps, perms


def kernel(x, Wq, bq, Wk, bk, Wv, bv):
    from concourse.bass_utils import run_bass_kernel_spmd

    nc = _get_module()
    in_maps, perms = _host_prep(x, Wq, bq, Wk, bk, Wv, bv)
    res = run_bass_kernel_spmd(
        nc,
        in_maps,
        core_ids=list(range(8)),
        trace=TRACE,
        **TRACE_KW,
    )
    _CACHE["last_result"] = res

    bv64 = np.asarray(bv, np.float64)
    out = np.empty((B, S, D), dtype=np.float32)
    for b in range(B):
        r0, r1 = res.results[2 * b], res.results[2 * b + 1]
        pv = np.zeros((D, S), dtype=np.float64)
        sm = np.zeros((S,), dtype=np.float64)
        for r, perm in ((r0, perms[2 * b]), (r1, perms[2 * b + 1])):
            pv[:, perm] += r["pvt"].astype(np.float64)
            sm[perm] += r["sums"][0].astype(np.float64)
        out[b] = ((pv / sm[None, :]).T + bv64[None, :]).astype(np.float32)
    return out


# revision 38
# speedup vs baseline: 1166.7505x; 1.0338x over previous
"""Causal self-attention (B=4, S=2048, E=1024, D=128, single head) on 8 TRN2 cores.

Sharding: core c = 2*b + h handles batch b; the two cores of a pair split the
causal key range by k-tile parity (h=0 even 128-row k-tiles, h=1 odd). All 8
cores run the *same* instruction stream; per-core differences live in DRAM
data only:
  - xt [1024, 2048] fp16: x[b].T with 128-col s-tiles stored in "perm order"
    (position p holds global tile p^h), so EVEN positions are always the
    core's own-parity tiles. K/V projection reads even positions via a
    strided AP; Q projection reads all positions in storage order.
  - mask [128, 256] fp16: additive score mask for the two diagonal-region
    k-tiles of every q-block ([tri | 0] for h=0, [tri | -6e4] for h=1).

Math shortcuts vs the reference:
  - K bias dropped entirely: softmax(q.(k+bk)) == softmax(q.k) since the
    q.bk term is constant across keys for a fixed query row.
  - V bias moved to the host: rows of normalized attention sum to 1, so
    out = (pv/sums) + bv exactly.
  - V is projected directly in NATURAL orientation (stationary = 128x128
    x-block, moving = Wv e-tile), killing the 8 PE transposes (~275ns each
    on HW) and the identity tensor.

Engine split (Pool/gpsimd has no PSUM port): PE matmuls; ACT exp + pv/sums
staging copies; DVE mask adds, exp-accumulation adds, and K/Q/V PSUM->SBUF
copies; Pool drives the output DMA ring. PV output is stored fp16
(pvt [128 d, 2048 q perm]); denominators stay f32.

Each core emits unnormalized PV partials and denominators (sums [1, 2048]);
the host un-permutes, combines the pair, and adds bv:
  out[b] = ((pv0 + pv1) / (s0 + s1)).T + bv

Measured: rel err 5.4e-4 on HW. A/Bs were decided with a corrected
TimelineSim (simcmp.py: fp16 matmul at 2 cols/cycle, transposes 275 ns,
DMA rescaled to the ~358 GB/s HBM line rate): the staged baseline sims at
35728 ns, this kernel at 31072 ns (-13%), via
  - V-natural projection + engine resplit (kills 8 PE transposes),
  - separate K/Q psum banks so Q-proj matmuls never wait on the K
    psum->SBUF copy (proj_split),
  - each block's denominator/staging emitted after the NEXT chunk's
    projections so the PE never stalls on the DVE accumulation tail
    (fin_delay),
  - xt chunk DMAs coalesced 8 -> 2 per chunk via a rearranged DRAM AP
    (dma_merge; ~24 fewer SP-queue instructions + semaphores), with chunk
    0 kept per-e-tile (dma_merge_j0=8) so the first projection starts as
    soon as e-tile 0 + wk land,
  - both V k-tiles of a chunk accumulated into ONE [P, 2, D] psum tile
    (still one bank) so a single PSUM->SBUF copy moves the pair and the
    second tile's matmuls never wait on the first tile's copy (-0.9us).
Rejected by the same estimator: exp-merging (paired 1024-wide exps into a
two-half accumulator; also blocked by the 8-bank PSUM budget), packing K
and V psum into one shared bank (over-serializes, +1.1us), PE p-state
warmup spam, Pool-engine accumulation adds, masked-tiles-last emission,
attention-one-chunk-behind pipelining, and staging/sums copies on DVE.
fp8 (DoubleRow) Q/K projection was rejected for accuracy (rel err 5.5e-2
on HW vs the 2e-2 gate).
"""

import os

os.environ.setdefault("MYCRO_LOCAL_CACHE", "1")

import numpy as np

B, S, E, D = 4, 2048, 1024, 128
P = 128
NT = S // P          # 16 global s-tiles per batch
LT = NT // 2         # 8 local (per-core) k-tiles
NQB = 4              # 512-wide query blocks
QBW = 512
NEB = E // P         # 8 e-tiles
SCALE = 1.0 / float(np.sqrt(D))
NEG = -60000.0       # fp16-representable; exp underflows to 0 in fp32

TRACE = False        # set by test.py for profiling runs
TRACE_KW = {}
PROBE_NO_XT = False  # timing probe: skip xt DMA (results wrong; perf only)

_CACHE = {}


def _build_module(reps=1, adds="dve", staging="act", vncopy="dve",
                  sc_bufs=3, v_bufs=1, pp_bufs=4, sum_eng="act",
                  fin_delay=True, masked_last=False, proj_split=True,
                  order_variant=0, dma_merge=2, dma_merge_j0=8,
                  attn_delay=False, ap_bufs=2):
    """adds: engine for exp-accumulation adds ('dve'|'pool'|'mix' = small
    masked-tile adds on Pool, rest DVE). staging: engine for pv/sums
    PSUM->SBUF copies. vncopy: engine for V-natural PSUM->SBUF copies."""
    from contextlib import ExitStack

    import concourse.bacc as bacc
    import concourse.mybir as mybir
    import concourse.tile as tile

    f32 = mybir.dt.float32
    f16 = mybir.dt.float16

    nc = bacc.Bacc("TRN2", target_bir_lowering=False, debug=False, num_devices=8)

    xt_d = nc.dram_tensor("xt", [E, S], f16, kind="ExternalInput").ap()
    wq_d = nc.dram_tensor("wq", [E, D], f16, kind="ExternalInput").ap()
    wk_d = nc.dram_tensor("wk", [E, D], f16, kind="ExternalInput").ap()
    wv_d = nc.dram_tensor("wv", [E, D], f16, kind="ExternalInput").ap()
    bq_d = nc.dram_tensor("bq", [D], f32, kind="ExternalInput").ap()  # pre-scaled
    mask_d = nc.dram_tensor("mask", [P, 2 * P], f16, kind="ExternalInput").ap()
    ones_d = nc.dram_tensor("ones", [P, 1], f16, kind="ExternalInput").ap()
    pvt_d = nc.dram_tensor("pvt", [D, S], f16, kind="ExternalOutput").ap()
    sums_d = nc.dram_tensor("sums", [1, S], f32, kind="ExternalOutput").ap()

    with tile.TileContext(nc) as tc, ExitStack() as ctx:
        singles = ctx.enter_context(tc.tile_pool(name="singles", bufs=1))
        ppool = ctx.enter_context(tc.tile_pool(name="ppool", bufs=pp_bufs))
        apool = ctx.enter_context(tc.tile_pool(name="apool", bufs=ap_bufs))
        proj_ps = ctx.enter_context(tc.tile_pool(name="proj_ps", bufs=1, space="PSUM"))
        kv_ps = ctx.enter_context(tc.tile_pool(name="kv_ps", bufs=v_bufs, space="PSUM"))
        sc_ps = ctx.enter_context(tc.tile_pool(name="sc_ps", bufs=sc_bufs, space="PSUM"))
        pv_ps = ctx.enter_context(tc.tile_pool(name="pv_ps", bufs=1, space="PSUM"))
        sum_ps = ctx.enter_context(tc.tile_pool(name="sum_ps", bufs=1, space="PSUM"))

        # ---- constants (ACT HWDGE ring; xt stream owns the SP ring) ----
        w_sb = {}
        for name, dram in (("wk", wk_d), ("wv", wv_d), ("wq", wq_d)):
            t = singles.tile([P, NEB, D], f16, tag=f"w_{name}")
            nc.scalar.dma_start(t[:], dram.rearrange("(o p) d -> p o d", p=P))
            w_sb[name] = t
        bq_sb = singles.tile([P, 1], f32, tag="b_bq")
        nc.scalar.dma_start(bq_sb[:], bq_d.rearrange("(p one) -> p one", one=1))
        mask_sb = singles.tile([P, 2 * P], f16, tag="mask")
        nc.scalar.dma_start(mask_sb[:], mask_d[:])
        ones = singles.tile([P, 1], f16, tag="ones")
        nc.scalar.dma_start(ones[:], ones_d[:])

        # ---- persistent activations ----
        # xt resident in perm order: [e-part, e-tile, pair, parity-pos, col]
        xt_sb = singles.tile([P, NEB, LT, 2, P], f16, tag="xt_sb")
        kt = singles.tile([P, LT, P], f16, tag="kt")      # K^T  [d, lt, k]
        vn = singles.tile([P, LT, D], f16, tag="vn")      # V natural [k, lt, d]
        qt = singles.tile([P, NT, P], f16, tag="qt")      # Q^T [d, pos, q]
        pvt_sb = singles.tile([D, S], f16, tag="pvt_sb")
        sums_sb = singles.tile([1, S], f32, tag="sums_sb")

        ktv = kt.rearrange("p lt k -> p (lt k)")

        xt_d3 = xt_d.rearrange("(o p) s -> p o s", p=P)

        def load_chunk(J, rep=0):
            """DMA xt columns [512J, 512J+512) (positions 4J..4J+3).

            nd = DMAs per chunk (1, 2, 4, or 8): fewer, bigger transfers cut
            SP-queue instruction + semaphore overhead; chunk 0 can stay
            fine-grained (dma_merge_j0) so the first projection starts after
            one e-tile instead of the whole 1 MB chunk."""
            if PROBE_NO_XT and rep > 0:
                return
            nd = dma_merge_j0 if (J == 0 and rep == 0) else dma_merge
            step = NEB // nd
            for g in range(nd):
                eo0 = g * step
                nc.sync.dma_start(
                    xt_sb[:, eo0 : eo0 + step, 2 * J : 2 * J + 2, :, :],
                    xt_d3[:, eo0 : eo0 + step, J * QBW : (J + 1) * QBW],
                )

        def proj_chunk(J):
            """K/V for local k-tiles {2J, 2J+1} + Q for block J."""
            # K^T: stationary Wk e-tile, moving x evens; no bias (see docstring)
            if proj_split:
                ps = kv_ps.tile([P, 2 * P], f32, tag="ps_k")
            else:
                ps = proj_ps.tile([P, QBW], f32, tag="ps_kq")
            for eo in range(NEB):
                nc.tensor.matmul(
                    ps[:, : 2 * P],
                    w_sb["wk"][:, eo, :],
                    xt_sb[:, eo, 2 * J : 2 * J + 2, 0, :],
                    start=(eo == 0),
                    stop=(eo == NEB - 1),
                )
            nc.vector.tensor_copy(
                out=ktv[:, J * 2 * P : (J + 1) * 2 * P], in_=ps[:, : 2 * P]
            )
            # V natural: stationary x-block [e, s-tile], moving Wv e-tile.
            # Both k-tiles accumulate into one [P, 2, D] psum tile (still one
            # bank) so a single PSUM->SBUF copy moves the pair and the second
            # tile's matmuls never wait on the first tile's copy.
            vps = kv_ps.tile([P, 2, D], f32, tag="ps_v")
            for vi, lt in enumerate((2 * J, 2 * J + 1)):
                for eo in range(NEB):
                    nc.tensor.matmul(
                        vps[:, vi, :],
                        xt_sb[:, eo, lt, 0, :],
                        w_sb["wv"][:, eo, :],
                        start=(eo == 0),
                        stop=(eo == NEB - 1),
                        skip_group_check=True,
                    )
            if vncopy == "dve":
                nc.vector.tensor_copy(
                    out=vn[:, 2 * J : 2 * J + 2, :], in_=vps[:]
                )
            else:
                nc.scalar.copy(out=vn[:, 2 * J : 2 * J + 2, :], in_=vps[:])
            # Q^T over all 4 positions of the chunk
            ps = proj_ps.tile([P, QBW], f32, tag="ps_q" if proj_split else "ps_kq")
            for eo in range(NEB):
                nc.tensor.matmul(
                    ps[:],
                    w_sb["wq"][:, eo, :],
                    xt_sb[:, eo, 2 * J : 2 * J + 2, :, :],
                    start=(eo == 0),
                    stop=(eo == NEB - 1),
                )
            qv = qt.rearrange("p t q -> p (t q)")
            nc.vector.tensor_scalar(
                qv[:, J * QBW : (J + 1) * QBW],
                ps[:],
                SCALE,
                bq_sb[:],
                mybir.AluOpType.mult,
                mybir.AluOpType.add,
            )

        def attention_blk(J):
            """Block J: q-cols [512J, 512J+512), local k-tiles 0..2J+1."""
            nlt = 2 * J + 2
            pv = pv_ps.tile([P, QBW], f32, tag="pv")
            acc = apool.tile([P, QBW], f16, tag="acc")
            # emission order: full-width i=0 first (clears PSUM over the
            # whole block), then the masked tiles (their DVE-mask + exp
            # latency hides behind the remaining scores matmuls), ending on
            # a plain tile so only one exp latency is exposed at block end.
            if J == 0:
                order = [0, 1]
            elif masked_last:
                order = [0] + list(range(1, 2 * J)) + [2 * J, 2 * J + 1]
            elif order_variant == 1 and J >= 1:
                order = [0, 2 * J, 1, 2 * J + 1] + list(range(2, 2 * J))
                order = [t for t in order if t < 2 * J + 2][:2 * J + 2]
                if J == 1:
                    order = [0, 2, 3]
            elif order_variant == 2 and J >= 1:
                order = [2 * J, 0, 2 * J + 1] + list(range(1, 2 * J))
            else:
                order = [0, 2 * J, 2 * J + 1] + list(range(1, 2 * J))
            for idx, i in enumerate(order):
                c0 = 2 * P if i == 2 * J + 1 else 0
                sc = sc_ps.tile([P, QBW], f32, tag="sc")
                nc.tensor.matmul(
                    sc[:, c0:],
                    kt[:, i, :],
                    qt[:, 4 * J + c0 // P : 4 * J + 4, :],
                    start=True,
                    stop=True,
                )
                if i >= 2 * J:
                    nc.vector.tensor_tensor(
                        out=sc[:, c0 : c0 + 2 * P],
                        in0=sc[:, c0 : c0 + 2 * P],
                        in1=mask_sb[:],
                        op=mybir.AluOpType.add,
                    )
                if idx == 0:
                    psrc = acc
                    nc.scalar.activation(
                        acc[:], sc[:], mybir.ActivationFunctionType.Exp
                    )
                else:
                    psrc = ppool.tile([P, QBW], f16, tag="p")
                    nc.scalar.activation(
                        psrc[:, c0:], sc[:, c0:], mybir.ActivationFunctionType.Exp
                    )
                    if adds == "mix":
                        add_eng = nc.gpsimd if c0 else nc.vector
                    else:
                        add_eng = nc.vector if adds == "dve" else nc.gpsimd
                    add_eng.tensor_tensor(
                        out=acc[:, c0:],
                        in0=acc[:, c0:],
                        in1=psrc[:, c0:],
                        op=mybir.AluOpType.add,
                    )
                nc.tensor.matmul(
                    pv[:, c0:],
                    vn[:, i, :],
                    psrc[:, c0:],
                    start=(idx == 0),
                    stop=(idx == nlt - 1),
                    skip_group_check=True,
                )
            return pv, acc

        def finish_blk(J, pv, acc):
            """Block J tail: denominator matmul, staging copies, output DMA."""
            col0 = J * QBW
            sm = sum_ps.tile([1, QBW], f32, tag="sm")
            nc.tensor.matmul(sm[:], ones[:], acc[:], start=True, stop=True)
            if staging == "act":
                nc.scalar.copy(out=pvt_sb[:, col0 : col0 + QBW], in_=pv[:])
            else:
                nc.vector.tensor_copy(out=pvt_sb[:, col0 : col0 + QBW], in_=pv[:])
            if sum_eng == "act":
                nc.scalar.copy(out=sums_sb[:, col0 : col0 + QBW], in_=sm[:])
            else:
                nc.vector.tensor_copy(out=sums_sb[:, col0 : col0 + QBW], in_=sm[:])
            out_eng = nc.gpsimd
            out_eng.dma_start(
                pvt_d[:, col0 : col0 + QBW], pvt_sb[:, col0 : col0 + QBW]
            )
            out_eng.dma_start(
                sums_d[:, col0 : col0 + QBW], sums_sb[:, col0 : col0 + QBW]
            )

        pend = None
        pend_attn = None
        for _rep in range(reps):
            for J in range(NQB):
                load_chunk(J, _rep)
                proj_chunk(J)
                if attn_delay:
                    if pend is not None:
                        finish_blk(*pend)
                        pend = None
                    if pend_attn is not None:
                        pend = (pend_attn, *attention_blk(pend_attn))
                    pend_attn = J
                    continue
                if not fin_delay:
                    finish_blk(J, *attention_blk(J))
                    continue
                if pend is not None:
                    finish_blk(*pend)
                pend = (J, *attention_blk(J))
        if pend is not None:
            finish_blk(*pend)
            pend = None
        if pend_attn is not None:
            finish_blk(pend_attn, *attention_blk(pend_attn))

    nc.compile()
    return nc


def _get_module(reps=1, **kw):
    key = ("nc", reps, tuple(sorted(kw.items())))
    if key not in _CACHE:
        _CACHE[key] = _build_module(reps, **kw)
    return _CACHE[key]


def _host_prep(x, Wq, bq, Wk, bk, Wv, bv):
    """Build the 8 per-core input maps plus per-core q-column permutations."""
    x = np.asarray(x, dtype=np.float32)
    tri = np.where(
        np.arange(P)[None, :] >= np.arange(P)[:, None], 0.0, NEG
    ).astype(np.float16)
    in_maps = []
    perms = []
    for c in range(8):
        b, h = divmod(c, 2)
        xt3 = np.ascontiguousarray(x[b].T).reshape(E, NT, P)
        # perm order: position p holds global tile p^h
        pos = np.arange(NT) ^ h
        xt_perm = np.ascontiguousarray(xt3[:, pos, :].reshape(E, S)).astype(
            np.float16
        )
        mask = np.concatenate(
            [tri, np.full((P, P), 0.0 if h == 0 else NEG, np.float16)], axis=1
        )
        in_maps.append(
            {
                "xt": xt_perm,
                "wq": np.asarray(Wq, np.float16),
                "wk": np.asarray(Wk, np.float16),
                "wv": np.asarray(Wv, np.float16),
                "bq": np.asarray(bq, np.float32) * np.float32(SCALE),
                "mask": np.ascontiguousarray(mask),
                "ones": np.ones((P, 1), dtype=np.float16),
            }
        )
        # storage col -> global q row (position tile p holds global tile p^h)
        perm = np.empty(S, dtype=np.int64)
        for t in range(NT):
            perm[t * P : (t + 1) * P] = (t ^ h) * P + np.arange(P)
        perms.append(perm)
    return in_maps, perms


def kernel(x, Wq, bq, Wk, bk, Wv, bv):
    from concourse.bass_utils import run_bass_kernel_spmd

    nc = _get_module()
    in_maps, perms = _host_prep(x, Wq, bq, Wk, bk, Wv, bv)
    res = run_bass_kernel_spmd(
        nc,
        in_maps,
        core_ids=list(range(8)),
        trace=TRACE,
        **TRACE_KW,
    )
    _CACHE["last_result"] = res

    bv64 = np.asarray(bv, np.float64)
    out = np.empty((B, S, D), dtype=np.float32)
    for b in range(B):
        r0, r1 = res.results[2 * b], res.results[2 * b + 1]
        pv = np.zeros((D, S), dtype=np.float64)
        sm = np.zeros((S,), dtype=np.float64)
        for r, perm in ((r0, perms[2 * b]), (r1, perms[2 * b + 1])):
            pv[:, perm] += r["pvt"].astype(np.float64)
            sm[perm] += r["sums"][0].astype(np.float64)
        out[b] = ((pv / sm[None, :]).T + bv64[None, :]).astype(np.float32)
    return out


# revision 52
# speedup vs baseline: 1185.6774x; 1.0162x over previous
"""Causal self-attention (B=4, S=2048, E=1024, D=128, single head) on 8 TRN2 cores.

Sharding: core c = 2*b + h handles batch b; the two cores of a pair split the
causal key range by k-tile parity (h=0 even 128-row k-tiles, h=1 odd). All 8
cores run the *same* instruction stream; per-core differences live in DRAM
data only:
  - xt [1024, 2048] fp16: x[b].T with 128-col s-tiles stored in "perm order"
    (position p holds global tile p^h), so EVEN positions are always the
    core's own-parity tiles. K/V projection reads even positions via a
    strided AP; Q projection reads all positions in storage order.
  - mask [128, 256] fp16: additive score mask for the two diagonal-region
    k-tiles of every q-block ([tri | 0] for h=0, [tri | -6e4] for h=1).

Math shortcuts vs the reference:
  - K bias dropped entirely: softmax(q.(k+bk)) == softmax(q.k) since the
    q.bk term is constant across keys for a fixed query row.
  - V bias moved to the host: rows of normalized attention sum to 1, so
    out = (pv/sums) + bv exactly.
  - V is projected directly in NATURAL orientation (stationary = 128x128
    x-block, moving = Wv e-tile), killing the 8 PE transposes (~275ns each
    on HW) and the identity tensor.

Engine split (Pool/gpsimd has no PSUM port): PE matmuls; ACT exp + pv/sums
staging copies; DVE mask adds, exp-accumulation adds, and K/Q/V PSUM->SBUF
copies; Pool drives the output DMA ring. PV output is stored fp16
(pvt [128 d, 2048 q perm]); denominators stay f32.

Each core emits unnormalized PV partials and denominators (sums [1, 2048]);
the host un-permutes, combines the pair, and adds bv:
  out[b] = ((pv0 + pv1) / (s0 + s1)).T + bv

Measured: rel err 5.4e-4 on HW. A/Bs were decided with a corrected
TimelineSim (simcmp.py: fp16 matmul at 2 cols/cycle, transposes 275 ns,
DMA rescaled to the ~358 GB/s HBM line rate): the staged baseline sims at
35728 ns, this kernel at 30576 ns (-14.4%), via
  - V-natural projection + engine resplit (kills 8 PE transposes),
  - separate K/Q psum banks so Q-proj matmuls never wait on the K
    psum->SBUF copy (proj_split),
  - each block's denominator/staging emitted after the NEXT chunk's
    projections so the PE never stalls on the DVE accumulation tail
    (fin_delay),
  - xt chunk DMAs coalesced 8 -> 2 per chunk via a rearranged DRAM AP
    (dma_merge; ~24 fewer SP-queue instructions + semaphores), with chunk
    0 kept per-e-tile (dma_merge_j0=8) so the first projection starts as
    soon as e-tile 0 + wk land,
  - both V k-tiles of a chunk accumulated into ONE [P, 2, D] psum tile
    (still one bank) so a single PSUM->SBUF copy moves the pair and the
    second tile's matmuls never wait on the first tile's copy (-0.9us),
  - output DMAs on the SP HWDGE ring (idle after the last chunk load)
    instead of the Pool SWDGE queue (-0.5us: no software descriptor
    generation on the output path).
Rejected by the same estimator: exp-merging (paired 1024-wide exps into a
two-half accumulator; also blocked by the 8-bank PSUM budget), packing K
and V psum into one shared bank (over-serializes, +1.1us), PE p-state
warmup spam, Pool-engine accumulation adds, masked-tiles-last emission,
attention-one-chunk-behind pipelining, and staging/sums copies on DVE.
fp8 (DoubleRow) Q/K projection was rejected for accuracy (rel err 5.5e-2
on HW vs the 2e-2 gate).
"""

import os

os.environ.setdefault("MYCRO_LOCAL_CACHE", "1")

import numpy as np

B, S, E, D = 4, 2048, 1024, 128
P = 128
NT = S // P          # 16 global s-tiles per batch
LT = NT // 2         # 8 local (per-core) k-tiles
NQB = 4              # 512-wide query blocks
QBW = 512
NEB = E // P         # 8 e-tiles
SCALE = 1.0 / float(np.sqrt(D))
NEG = -60000.0       # fp16-representable; exp underflows to 0 in fp32

TRACE = False        # set by test.py for profiling runs
TRACE_KW = {}
PROBE_NO_XT = False  # timing probe: skip xt DMA (results wrong; perf only)

_CACHE = {}


def _build_module(reps=1, adds="dve", staging="act", vncopy="dve",
                  sc_bufs=3, v_bufs=1, pp_bufs=4, sum_eng="act",
                  fin_delay=True, masked_last=False, proj_split=True,
                  order_variant=0, dma_merge=2, dma_merge_j0=8,
                  attn_delay=False, ap_bufs=2, split_attn=False, warmup=0,
                  evf=False, ring_split=False, out_ring="sp", wsplit=False):
    """adds: engine for exp-accumulation adds ('dve'|'pool'|'mix' = small
    masked-tile adds on Pool, rest DVE). staging: engine for pv/sums
    PSUM->SBUF copies. vncopy: engine for V-natural PSUM->SBUF copies."""
    from contextlib import ExitStack

    import concourse.bacc as bacc
    import concourse.mybir as mybir
    import concourse.tile as tile

    f32 = mybir.dt.float32
    f16 = mybir.dt.float16

    nc = bacc.Bacc("TRN2", target_bir_lowering=False, debug=False, num_devices=8)

    xt_d = nc.dram_tensor("xt", [E, S], f16, kind="ExternalInput").ap()
    wq_d = nc.dram_tensor("wq", [E, D], f16, kind="ExternalInput").ap()
    wk_d = nc.dram_tensor("wk", [E, D], f16, kind="ExternalInput").ap()
    wv_d = nc.dram_tensor("wv", [E, D], f16, kind="ExternalInput").ap()
    bq_d = nc.dram_tensor("bq", [D], f32, kind="ExternalInput").ap()  # pre-scaled
    mask_d = nc.dram_tensor("mask", [P, 2 * P], f16, kind="ExternalInput").ap()
    maskw_d = (nc.dram_tensor("maskw", [P, 4 * P], f16, kind="ExternalInput").ap()
               if evf else None)
    ones_d = nc.dram_tensor("ones", [P, 1], f16, kind="ExternalInput").ap()
    pvt_d = nc.dram_tensor("pvt", [D, S], f16, kind="ExternalOutput").ap()
    sums_d = nc.dram_tensor("sums", [1, S], f32, kind="ExternalOutput").ap()

    with tile.TileContext(nc) as tc, ExitStack() as ctx:
        singles = ctx.enter_context(tc.tile_pool(name="singles", bufs=1))
        ppool = ctx.enter_context(tc.tile_pool(name="ppool", bufs=pp_bufs))
        apool = ctx.enter_context(tc.tile_pool(name="apool", bufs=ap_bufs))
        proj_ps = ctx.enter_context(tc.tile_pool(name="proj_ps", bufs=1, space="PSUM"))
        kv_ps = ctx.enter_context(tc.tile_pool(name="kv_ps", bufs=v_bufs, space="PSUM"))
        sc_ps = ctx.enter_context(tc.tile_pool(name="sc_ps", bufs=sc_bufs, space="PSUM"))
        pv_ps = ctx.enter_context(tc.tile_pool(name="pv_ps", bufs=1, space="PSUM"))
        sum_ps = ctx.enter_context(tc.tile_pool(name="sum_ps", bufs=1, space="PSUM"))

        # ---- constants (ACT HWDGE ring; xt stream owns the SP ring) ----
        w_sb = {}
        for name, dram, nsplit in (("wk", wk_d, 4), ("wv", wv_d, 1),
                                   ("wq", wq_d, 1)):
            t = singles.tile([P, NEB, D], f16, tag=f"w_{name}")
            d3 = dram.rearrange("(o p) d -> p o d", p=P)
            if not wsplit:
                nsplit = 1
            step = NEB // nsplit
            for g in range(nsplit):
                nc.scalar.dma_start(
                    t[:, g * step : (g + 1) * step, :],
                    d3[:, g * step : (g + 1) * step, :],
                )
            w_sb[name] = t
        bq_sb = singles.tile([P, 1], f32, tag="b_bq")
        nc.scalar.dma_start(bq_sb[:], bq_d.rearrange("(p one) -> p one", one=1))
        mask_sb = singles.tile([P, 2 * P], f16, tag="mask")
        nc.scalar.dma_start(mask_sb[:], mask_d[:])
        maskw_sb = None
        if evf:
            maskw_sb = singles.tile([P, 4 * P], f16, tag="maskw")
            nc.scalar.dma_start(maskw_sb[:], maskw_d[:])
        ones = singles.tile([P, 1], f16, tag="ones")
        nc.scalar.dma_start(ones[:], ones_d[:])

        # ---- persistent activations ----
        # xt resident in perm order: [e-part, e-tile, pair, parity-pos, col]
        if evf:
            # slot order per chunk: [4J, 4J+2, 4J+3, 4J+1] (evens | odds rev)
            xt_sb = singles.tile([P, NEB, NQB, 4, P], f16, tag="xt_sb")
        else:
            xt_sb = singles.tile([P, NEB, LT, 2, P], f16, tag="xt_sb")
        kt = singles.tile([P, LT, P], f16, tag="kt")      # K^T  [d, lt, k]
        vn = singles.tile([P, LT, D], f16, tag="vn")      # V natural [k, lt, d]
        qt = singles.tile([P, NT, P], f16, tag="qt")      # Q^T [d, pos, q]
        pvt_sb = singles.tile([D, S], f16, tag="pvt_sb")
        sums_sb = singles.tile([1, S], f32, tag="sums_sb")

        ktv = kt.rearrange("p lt k -> p (lt k)")

        if warmup:
            wps = sum_ps.tile([1, QBW], f32, tag="sm")
            for _w in range(warmup):
                nc.tensor.matmul(
                    wps[:, 0:64],
                    ones[:, 0:1],
                    mask_sb[:, 0:64],
                    start=True,
                    stop=True,
                    skip_group_check=True,
                )

        xt_d3 = xt_d.rearrange("(o p) s -> p o s", p=P)

        def load_chunk(J, rep=0):
            """DMA xt columns [512J, 512J+512) (positions 4J..4J+3).

            nd = DMAs per chunk (1, 2, 4, or 8): fewer, bigger transfers cut
            SP-queue instruction + semaphore overhead; chunk 0 can stay
            fine-grained (dma_merge_j0) so the first projection starts after
            one e-tile instead of the whole 1 MB chunk."""
            if PROBE_NO_XT and rep > 0:
                return
            nd = dma_merge_j0 if (J == 0 and rep == 0) else dma_merge
            if evf:
                # evens fine-grained (K/V proj starts early), odds coarse
                for half, nh in ((0, max(nd // 2, 2)), (1, 1)):
                    c0 = J * QBW + half * 2 * P
                    step = NEB // nh
                    for g in range(nh):
                        eo0 = g * step
                        nc.sync.dma_start(
                            xt_sb[:, eo0 : eo0 + step, J,
                                  2 * half : 2 * half + 2, :],
                            xt_d3[:, eo0 : eo0 + step, c0 : c0 + 2 * P],
                        )
                return
            step = NEB // nd
            xt_eng = nc.scalar if (ring_split and J % 2 == 1) else nc.sync
            for g in range(nd):
                eo0 = g * step
                xt_eng.dma_start(
                    xt_sb[:, eo0 : eo0 + step, 2 * J : 2 * J + 2, :, :],
                    xt_d3[:, eo0 : eo0 + step, J * QBW : (J + 1) * QBW],
                )

        def x_evens(eo, J):
            if evf:
                return xt_sb[:, eo, J, 0:2, :]
            return xt_sb[:, eo, 2 * J : 2 * J + 2, 0, :]

        def x_block(eo, J, vi):
            if evf:
                return xt_sb[:, eo, J, vi, :]
            return xt_sb[:, eo, 2 * J + vi, 0, :]

        def x_all(eo, J):
            if evf:
                return xt_sb[:, eo, J, :, :]
            return xt_sb[:, eo, 2 * J : 2 * J + 2, :, :]

        def proj_chunk_kv(J):
            """K/V for local k-tiles {2J, 2J+1}."""
            # K^T: stationary Wk e-tile, moving x evens; no bias (see docstring)
            if proj_split:
                ps = kv_ps.tile([P, 2 * P], f32, tag="ps_k")
            else:
                ps = proj_ps.tile([P, QBW], f32, tag="ps_kq")
            for eo in range(NEB):
                nc.tensor.matmul(
                    ps[:, : 2 * P],
                    w_sb["wk"][:, eo, :],
                    x_evens(eo, J),
                    start=(eo == 0),
                    stop=(eo == NEB - 1),
                )
            nc.vector.tensor_copy(
                out=ktv[:, J * 2 * P : (J + 1) * 2 * P], in_=ps[:, : 2 * P]
            )
            # V natural: stationary x-block [e, s-tile], moving Wv e-tile.
            # Both k-tiles accumulate into one [P, 2, D] psum tile (still one
            # bank) so a single PSUM->SBUF copy moves the pair and the second
            # tile's matmuls never wait on the first tile's copy.
            vps = kv_ps.tile([P, 2, D], f32, tag="ps_v")
            for vi, lt in enumerate((2 * J, 2 * J + 1)):
                for eo in range(NEB):
                    nc.tensor.matmul(
                        vps[:, vi, :],
                        x_block(eo, J, vi),
                        w_sb["wv"][:, eo, :],
                        start=(eo == 0),
                        stop=(eo == NEB - 1),
                        skip_group_check=True,
                    )
            if vncopy == "dve":
                nc.vector.tensor_copy(
                    out=vn[:, 2 * J : 2 * J + 2, :], in_=vps[:]
                )
            else:
                nc.scalar.copy(out=vn[:, 2 * J : 2 * J + 2, :], in_=vps[:])
        def proj_chunk_q(J):
            # Q^T over all 4 positions of the chunk
            ps = proj_ps.tile([P, QBW], f32, tag="ps_q" if proj_split else "ps_kq")
            for eo in range(NEB):
                nc.tensor.matmul(
                    ps[:],
                    w_sb["wq"][:, eo, :],
                    x_all(eo, J),
                    start=(eo == 0),
                    stop=(eo == NEB - 1),
                )
            qv = qt.rearrange("p t q -> p (t q)")
            nc.vector.tensor_scalar(
                qv[:, J * QBW : (J + 1) * QBW],
                ps[:],
                SCALE,
                bq_sb[:],
                mybir.AluOpType.mult,
                mybir.AluOpType.add,
            )

        def proj_chunk(J):
            proj_chunk_kv(J)
            proj_chunk_q(J)

        def attention_tiles(J, order, pv, acc, idx0, nlt):
            """Emit score/exp/add/PV for the given k-tiles of block J.
            idx0 = number of tiles of this block already emitted."""
            for off, i in enumerate(order):
                idx = idx0 + off
                if evf:
                    # slot order [4J, 4J+2, 4J+3, 4J+1]: diagonal tile 2J+1
                    # sees exactly slots 1..2 -> contiguous [P, 3P)
                    lo, hi = (P, 3 * P) if i == 2 * J + 1 else (0, QBW)
                else:
                    lo, hi = (2 * P, QBW) if i == 2 * J + 1 else (0, QBW)
                sc = sc_ps.tile([P, QBW], f32, tag="sc")
                nc.tensor.matmul(
                    sc[:, lo:hi],
                    kt[:, i, :],
                    qt[:, 4 * J + lo // P : 4 * J + hi // P, :],
                    start=True,
                    stop=True,
                )
                if i == 2 * J and evf:
                    nc.vector.tensor_tensor(
                        out=sc[:, 0:QBW],
                        in0=sc[:, 0:QBW],
                        in1=maskw_sb[:],
                        op=mybir.AluOpType.add,
                    )
                elif i >= 2 * J:
                    m0 = lo if evf else (0 if i == 2 * J else 2 * P)
                    nc.vector.tensor_tensor(
                        out=sc[:, m0 : m0 + 2 * P],
                        in0=sc[:, m0 : m0 + 2 * P],
                        in1=mask_sb[:],
                        op=mybir.AluOpType.add,
                    )
                if idx == 0:
                    psrc = acc
                    nc.scalar.activation(
                        acc[:], sc[:], mybir.ActivationFunctionType.Exp
                    )
                else:
                    psrc = ppool.tile([P, QBW], f16, tag="p")
                    nc.scalar.activation(
                        psrc[:, lo:hi], sc[:, lo:hi],
                        mybir.ActivationFunctionType.Exp,
                    )
                    if adds == "mix":
                        add_eng = nc.gpsimd if lo else nc.vector
                    else:
                        add_eng = nc.vector if adds == "dve" else nc.gpsimd
                    add_eng.tensor_tensor(
                        out=acc[:, lo:hi],
                        in0=acc[:, lo:hi],
                        in1=psrc[:, lo:hi],
                        op=mybir.AluOpType.add,
                    )
                nc.tensor.matmul(
                    pv[:, lo:hi],
                    vn[:, i, :],
                    psrc[:, lo:hi],
                    start=(idx == 0),
                    stop=(idx == nlt - 1),
                    skip_group_check=True,
                )

        def attention_blk(J):
            """Block J: q-cols [512J, 512J+512), local k-tiles 0..2J+1."""
            nlt = 2 * J + 2
            pv = pv_ps.tile([P, QBW], f32, tag="pv")
            acc = apool.tile([P, QBW], f16, tag="acc")
            # emission order: full-width i=0 first (clears PSUM over the
            # whole block), then the masked tiles (their DVE-mask + exp
            # latency hides behind the remaining scores matmuls), ending on
            # a plain tile so only one exp latency is exposed at block end.
            if J == 0:
                order = [0, 1]
            elif masked_last:
                order = [0] + list(range(1, 2 * J)) + [2 * J, 2 * J + 1]
            elif order_variant == 1 and J >= 1:
                order = [0, 2 * J, 1, 2 * J + 1] + list(range(2, 2 * J))
                order = [t for t in order if t < 2 * J + 2][:2 * J + 2]
                if J == 1:
                    order = [0, 2, 3]
            elif order_variant == 2 and J >= 1:
                order = [2 * J, 0, 2 * J + 1] + list(range(1, 2 * J))
            else:
                order = [0, 2 * J, 2 * J + 1] + list(range(1, 2 * J))
            attention_tiles(J, order, pv, acc, 0, nlt)
            return pv, acc

        def finish_blk(J, pv, acc):
            """Block J tail: denominator matmul, staging copies, output DMA."""
            col0 = J * QBW
            sm = sum_ps.tile([1, QBW], f32, tag="sm")
            nc.tensor.matmul(sm[:], ones[:], acc[:], start=True, stop=True)
            if staging == "act":
                nc.scalar.copy(out=pvt_sb[:, col0 : col0 + QBW], in_=pv[:])
            else:
                nc.vector.tensor_copy(out=pvt_sb[:, col0 : col0 + QBW], in_=pv[:])
            if sum_eng == "act":
                nc.scalar.copy(out=sums_sb[:, col0 : col0 + QBW], in_=sm[:])
            else:
                nc.vector.tensor_copy(out=sums_sb[:, col0 : col0 + QBW], in_=sm[:])
            out_eng = {"pool": nc.gpsimd, "act": nc.scalar,
                       "sp": nc.sync}[out_ring]
            out_eng.dma_start(
                pvt_d[:, col0 : col0 + QBW], pvt_sb[:, col0 : col0 + QBW]
            )
            out_eng.dma_start(
                sums_d[:, col0 : col0 + QBW], sums_sb[:, col0 : col0 + QBW]
            )

        pend = None
        pend_attn = None
        for _rep in range(reps):
            for J in range(NQB):
                load_chunk(J, _rep)
                if split_attn:
                    # Q first, then attention on the k-tiles that already
                    # exist (0..2J-1) BETWEEN Q-proj and K/V-proj, so the
                    # exp/add chains overlap the projection matmuls; the two
                    # new (masked, diagonal) tiles run after K/V lands.
                    proj_chunk_q(J)
                    if pend is not None:
                        finish_blk(*pend)
                        pend = None
                    nlt = 2 * J + 2
                    pv = pv_ps.tile([P, QBW], f32, tag="pv")
                    acc = apool.tile([P, QBW], f16, tag="acc")
                    part_a = list(range(0, max(0, 2 * J - 1)))
                    part_b = [2 * J, 2 * J + 1] + ([2 * J - 1] if J >= 1 else [])
                    attention_tiles(J, part_a, pv, acc, 0, nlt)
                    proj_chunk_kv(J)
                    attention_tiles(J, part_b, pv, acc, len(part_a), nlt)
                    if fin_delay:
                        pend = (J, pv, acc)
                    else:
                        finish_blk(J, pv, acc)
                    continue
                proj_chunk(J)
                if attn_delay:
                    if pend is not None:
                        finish_blk(*pend)
                        pend = None
                    if pend_attn is not None:
                        pend = (pend_attn, *attention_blk(pend_attn))
                    pend_attn = J
                    continue
                if not fin_delay:
                    finish_blk(J, *attention_blk(J))
                    continue
                if pend is not None:
                    finish_blk(*pend)
                pend = (J, *attention_blk(J))
        if pend is not None:
            finish_blk(*pend)
            pend = None
        if pend_attn is not None:
            finish_blk(pend_attn, *attention_blk(pend_attn))

    nc.compile()
    return nc


def _get_module(reps=1, **kw):
    key = ("nc", reps, tuple(sorted(kw.items())))
    if key not in _CACHE:
        _CACHE[key] = _build_module(reps, **kw)
    return _CACHE[key]


def _host_prep(x, Wq, bq, Wk, bk, Wv, bv, evf=False):
    """Build the 8 per-core input maps plus per-core q-column permutations.

    evf: evens-first slot order per chunk [4J, 4J+2, 4J+3, 4J+1] (own-parity
    tiles first so K/V projection can start after a half-chunk DMA; odds
    reversed so the diagonal tile 2J+1 sees contiguous slots 1..2)."""
    x = np.asarray(x, dtype=np.float32)
    tri = np.where(
        np.arange(P)[None, :] >= np.arange(P)[:, None], 0.0, NEG
    ).astype(np.float16)
    in_maps = []
    perms = []
    for c in range(8):
        b, h = divmod(c, 2)
        xt3 = np.ascontiguousarray(x[b].T).reshape(E, NT, P)
        if evf:
            perm4 = np.array([0, 2, 3, 1])
            slot_to_global = np.concatenate(
                [(4 * J + perm4) ^ h for J in range(NQB)]
            )
        else:
            slot_to_global = np.arange(NT) ^ h
        xt_perm = np.ascontiguousarray(
            xt3[:, slot_to_global, :].reshape(E, S)
        ).astype(np.float16)
        mask = np.concatenate(
            [tri, np.full((P, P), 0.0 if h == 0 else NEG, np.float16)], axis=1
        )
        m = {
            "xt": xt_perm,
            "wq": np.asarray(Wq, np.float16),
            "wk": np.asarray(Wk, np.float16),
            "wv": np.asarray(Wv, np.float16),
            "bq": np.asarray(bq, np.float32) * np.float32(SCALE),
            "mask": np.ascontiguousarray(mask),
            "ones": np.ones((P, 1), dtype=np.float16),
        }
        if evf:
            m["maskw"] = np.ascontiguousarray(np.concatenate(
                [tri, np.zeros((P, 2 * P), np.float16),
                 np.full((P, P), 0.0 if h == 0 else NEG, np.float16)], axis=1
            ))
        in_maps.append(m)
        # storage col -> global q row
        perm = np.empty(S, dtype=np.int64)
        for t in range(NT):
            perm[t * P : (t + 1) * P] = slot_to_global[t] * P + np.arange(P)
        perms.append(perm)
    return in_maps, perms


EVF = False  # evens-first layout (see _host_prep)


def kernel(x, Wq, bq, Wk, bk, Wv, bv):
    from concourse.bass_utils import run_bass_kernel_spmd

    nc = _get_module(evf=EVF)
    in_maps, perms = _host_prep(x, Wq, bq, Wk, bk, Wv, bv, evf=EVF)
    res = run_bass_kernel_spmd(
        nc,
        in_maps,
        core_ids=list(range(8)),
        trace=TRACE,
        **TRACE_KW,
    )
    _CACHE["last_result"] = res

    bv64 = np.asarray(bv, np.float64)
    out = np.empty((B, S, D), dtype=np.float32)
    for b in range(B):
        r0, r1 = res.results[2 * b], res.results[2 * b + 1]
        pv = np.zeros((D, S), dtype=np.float64)
        sm = np.zeros((S,), dtype=np.float64)
        for r, perm in ((r0, perms[2 * b]), (r1, perms[2 * b + 1])):
            pv[:, perm] += r["pvt"].astype(np.float64)
            sm[perm] += r["sums"][0].astype(np.float64)
        out[b] = ((pv / sm[None, :]).T + bv64[None, :]).astype(np.float32)
    return out


# revision 57
# speedup vs baseline: 1193.4840x; 1.0066x over previous
"""Causal self-attention (B=4, S=2048, E=1024, D=128, single head) on 8 TRN2 cores.

Sharding: core c = 2*b + h handles batch b; the two cores of a pair split the
causal key range by k-tile parity (h=0 even 128-row k-tiles, h=1 odd). All 8
cores run the *same* instruction stream; per-core differences live in DRAM
data only:
  - xt [1024, 2048] fp16: x[b].T with 128-col s-tiles stored in "perm order"
    (position p holds global tile p^h), so EVEN positions are always the
    core's own-parity tiles. K/V projection reads even positions via a
    strided AP; Q projection reads all positions in storage order.
  - mask [128, 256] fp16: additive score mask for the two diagonal-region
    k-tiles of every q-block ([tri | 0] for h=0, [tri | -6e4] for h=1).

Math shortcuts vs the reference:
  - K bias dropped entirely: softmax(q.(k+bk)) == softmax(q.k) since the
    q.bk term is constant across keys for a fixed query row.
  - V bias moved to the host: rows of normalized attention sum to 1, so
    out = (pv/sums) + bv exactly.
  - V is projected directly in NATURAL orientation (stationary = 128x128
    x-block, moving = Wv e-tile), killing the 8 PE transposes (~275ns each
    on HW) and the identity tensor.

Engine split (Pool/gpsimd has no PSUM port): PE matmuls; ACT exp + pv/sums
staging copies; DVE mask adds, exp-accumulation adds, and K/Q/V PSUM->SBUF
copies; Pool drives the output DMA ring. PV output is stored fp16
(pvt [128 d, 2048 q perm]); denominators stay f32.

Each core emits unnormalized PV partials and denominators (sums [1, 2048]);
the host un-permutes, combines the pair, and adds bv:
  out[b] = ((pv0 + pv1) / (s0 + s1)).T + bv

Measured: rel err 5.4e-4 on HW. A/Bs were decided with a corrected
TimelineSim (simcmp.py: fp16 matmul at 2 cols/cycle, transposes 275 ns,
DMA rescaled to the ~358 GB/s HBM line rate): the staged baseline sims at
35728 ns, this kernel at 30376 ns (-15%), via
  - V-natural projection + engine resplit (kills 8 PE transposes),
  - separate K/Q psum banks so Q-proj matmuls never wait on the K
    psum->SBUF copy (proj_split),
  - each block's denominator/staging emitted after the NEXT chunk's
    projections so the PE never stalls on the DVE accumulation tail
    (fin_delay),
  - xt chunk DMAs coalesced 8 -> 2 per chunk via a rearranged DRAM AP
    (dma_merge; ~24 fewer SP-queue instructions + semaphores), with chunk
    0 kept per-e-tile (dma_merge_j0=8) so the first projection starts as
    soon as e-tile 0 + wk land,
  - both V k-tiles of a chunk accumulated into ONE [P, 2, D] psum tile
    (still one bank) so a single PSUM->SBUF copy moves the pair and the
    second tile's matmuls never wait on the first tile's copy (-0.9us),
  - output DMAs on the SP HWDGE ring (idle after the last chunk load)
    instead of the Pool SWDGE queue (-0.5us: no software descriptor
    generation on the output path), with blocks 0..2 shipped as one
    merged pvt + one sums transfer once block 2 is staged (out_merge,
    -0.2us; block 3 stays separate so the exit tail doesn't grow).
Rejected by the same estimator: exp-merging (paired 1024-wide exps into a
two-half accumulator; also blocked by the 8-bank PSUM budget), packing K
and V psum into one shared bank (over-serializes, +1.1us), PE p-state
warmup spam, Pool-engine accumulation adds, masked-tiles-last emission,
attention-one-chunk-behind pipelining, and staging/sums copies on DVE.
fp8 (DoubleRow) Q/K projection was rejected for accuracy (rel err 5.5e-2
on HW vs the 2e-2 gate).
"""

import os

os.environ.setdefault("MYCRO_LOCAL_CACHE", "1")

import numpy as np

B, S, E, D = 4, 2048, 1024, 128
P = 128
NT = S // P          # 16 global s-tiles per batch
LT = NT // 2         # 8 local (per-core) k-tiles
NQB = 4              # 512-wide query blocks
QBW = 512
NEB = E // P         # 8 e-tiles
SCALE = 1.0 / float(np.sqrt(D))
NEG = -60000.0       # fp16-representable; exp underflows to 0 in fp32

TRACE = False        # set by test.py for profiling runs
TRACE_KW = {}
PROBE_NO_XT = False  # timing probe: skip xt DMA (results wrong; perf only)

_CACHE = {}


def _build_module(reps=1, adds="dve", staging="act", vncopy="dve",
                  sc_bufs=3, v_bufs=1, pp_bufs=4, sum_eng="act",
                  fin_delay=True, masked_last=False, proj_split=True,
                  order_variant=0, dma_merge=2, dma_merge_j0=8,
                  attn_delay=False, ap_bufs=2, split_attn=False, warmup=0,
                  evf=False, ring_split=False, out_ring="sp", wsplit=False,
                  out_merge=True):
    """adds: engine for exp-accumulation adds ('dve'|'pool'|'mix' = small
    masked-tile adds on Pool, rest DVE). staging: engine for pv/sums
    PSUM->SBUF copies. vncopy: engine for V-natural PSUM->SBUF copies."""
    from contextlib import ExitStack

    import concourse.bacc as bacc
    import concourse.mybir as mybir
    import concourse.tile as tile

    f32 = mybir.dt.float32
    f16 = mybir.dt.float16

    nc = bacc.Bacc("TRN2", target_bir_lowering=False, debug=False, num_devices=8)

    xt_d = nc.dram_tensor("xt", [E, S], f16, kind="ExternalInput").ap()
    wq_d = nc.dram_tensor("wq", [E, D], f16, kind="ExternalInput").ap()
    wk_d = nc.dram_tensor("wk", [E, D], f16, kind="ExternalInput").ap()
    wv_d = nc.dram_tensor("wv", [E, D], f16, kind="ExternalInput").ap()
    bq_d = nc.dram_tensor("bq", [D], f32, kind="ExternalInput").ap()  # pre-scaled
    mask_d = nc.dram_tensor("mask", [P, 2 * P], f16, kind="ExternalInput").ap()
    ones_d = nc.dram_tensor("ones", [P, 1], f16, kind="ExternalInput").ap()
    maskw_d = (nc.dram_tensor("maskw", [P, 4 * P], f16, kind="ExternalInput").ap()
               if evf else None)
    pvt_d = nc.dram_tensor("pvt", [D, S], f16, kind="ExternalOutput").ap()
    sums_d = nc.dram_tensor("sums", [1, S], f32, kind="ExternalOutput").ap()

    with tile.TileContext(nc) as tc, ExitStack() as ctx:
        singles = ctx.enter_context(tc.tile_pool(name="singles", bufs=1))
        ppool = ctx.enter_context(tc.tile_pool(name="ppool", bufs=pp_bufs))
        apool = ctx.enter_context(tc.tile_pool(name="apool", bufs=ap_bufs))
        proj_ps = ctx.enter_context(tc.tile_pool(name="proj_ps", bufs=1, space="PSUM"))
        kv_ps = ctx.enter_context(tc.tile_pool(name="kv_ps", bufs=v_bufs, space="PSUM"))
        sc_ps = ctx.enter_context(tc.tile_pool(name="sc_ps", bufs=sc_bufs, space="PSUM"))
        pv_ps = ctx.enter_context(tc.tile_pool(name="pv_ps", bufs=1, space="PSUM"))
        sum_ps = ctx.enter_context(tc.tile_pool(name="sum_ps", bufs=1, space="PSUM"))

        # ---- constants (ACT HWDGE ring; xt stream owns the SP ring) ----
        w_sb = {}
        for name, dram, nsplit in (("wk", wk_d, 4), ("wv", wv_d, 1),
                                   ("wq", wq_d, 1)):
            t = singles.tile([P, NEB, D], f16, tag=f"w_{name}")
            d3 = dram.rearrange("(o p) d -> p o d", p=P)
            if not wsplit:
                nsplit = 1
            step = NEB // nsplit
            for g in range(nsplit):
                nc.scalar.dma_start(
                    t[:, g * step : (g + 1) * step, :],
                    d3[:, g * step : (g + 1) * step, :],
                )
            w_sb[name] = t
        bq_sb = singles.tile([P, 1], f32, tag="b_bq")
        nc.scalar.dma_start(bq_sb[:], bq_d.rearrange("(p one) -> p one", one=1))
        mask_sb = singles.tile([P, 2 * P], f16, tag="mask")
        nc.scalar.dma_start(mask_sb[:], mask_d[:])
        ones = singles.tile([P, 1], f16, tag="ones")
        nc.scalar.dma_start(ones[:], ones_d[:])
        maskw_sb = None
        if evf:
            maskw_sb = singles.tile([P, 4 * P], f16, tag="maskw")
            nc.scalar.dma_start(maskw_sb[:], maskw_d[:])


        # ---- persistent activations ----
        # xt resident in perm order: [e-part, e-tile, pair, parity-pos, col]
        if evf:
            # slot order per chunk: [4J, 4J+2, 4J+3, 4J+1] (evens | odds rev)
            xt_sb = singles.tile([P, NEB, NQB, 4, P], f16, tag="xt_sb")
        else:
            xt_sb = singles.tile([P, NEB, LT, 2, P], f16, tag="xt_sb")
        kt = singles.tile([P, LT, P], f16, tag="kt")      # K^T  [d, lt, k]
        vn = singles.tile([P, LT, D], f16, tag="vn")      # V natural [k, lt, d]
        qt = singles.tile([P, NT, P], f16, tag="qt")      # Q^T [d, pos, q]
        pvt_sb = singles.tile([D, S], f16, tag="pvt_sb")
        sums_sb = singles.tile([1, S], f32, tag="sums_sb")

        ktv = kt.rearrange("p lt k -> p (lt k)")

        if warmup:
            wps = sum_ps.tile([1, QBW], f32, tag="sm")
            for _w in range(warmup):
                nc.tensor.matmul(
                    wps[:, 0:64],
                    ones[:, 0:1],
                    mask_sb[:, 0:64],
                    start=True,
                    stop=True,
                    skip_group_check=True,
                )

        xt_d3 = xt_d.rearrange("(o p) s -> p o s", p=P)

        def load_chunk(J, rep=0):
            """DMA xt columns [512J, 512J+512) (positions 4J..4J+3).

            nd = DMAs per chunk (1, 2, 4, or 8): fewer, bigger transfers cut
            SP-queue instruction + semaphore overhead; chunk 0 can stay
            fine-grained (dma_merge_j0) so the first projection starts after
            one e-tile instead of the whole 1 MB chunk."""
            if PROBE_NO_XT and rep > 0:
                return
            nd = dma_merge_j0 if (J == 0 and rep == 0) else dma_merge
            if evf:
                # evens fine-grained (K/V proj starts early), odds coarse
                for half, nh in ((0, max(nd // 2, 2)), (1, 1)):
                    c0 = J * QBW + half * 2 * P
                    step = NEB // nh
                    for g in range(nh):
                        eo0 = g * step
                        nc.sync.dma_start(
                            xt_sb[:, eo0 : eo0 + step, J,
                                  2 * half : 2 * half + 2, :],
                            xt_d3[:, eo0 : eo0 + step, c0 : c0 + 2 * P],
                        )
                return
            step = NEB // nd
            xt_eng = nc.scalar if (ring_split and J % 2 == 1) else nc.sync
            for g in range(nd):
                eo0 = g * step
                xt_eng.dma_start(
                    xt_sb[:, eo0 : eo0 + step, 2 * J : 2 * J + 2, :, :],
                    xt_d3[:, eo0 : eo0 + step, J * QBW : (J + 1) * QBW],
                )

        def x_evens(eo, J):
            if evf:
                return xt_sb[:, eo, J, 0:2, :]
            return xt_sb[:, eo, 2 * J : 2 * J + 2, 0, :]

        def x_block(eo, J, vi):
            if evf:
                return xt_sb[:, eo, J, vi, :]
            return xt_sb[:, eo, 2 * J + vi, 0, :]

        def x_all(eo, J):
            if evf:
                return xt_sb[:, eo, J, :, :]
            return xt_sb[:, eo, 2 * J : 2 * J + 2, :, :]

        def proj_chunk_kv(J):
            """K/V for local k-tiles {2J, 2J+1}."""
            # K^T: stationary Wk e-tile, moving x evens; no bias (see docstring)
            if proj_split:
                ps = kv_ps.tile([P, 2 * P], f32, tag="ps_k")
            else:
                ps = proj_ps.tile([P, QBW], f32, tag="ps_kq")
            for eo in range(NEB):
                nc.tensor.matmul(
                    ps[:, : 2 * P],
                    w_sb["wk"][:, eo, :],
                    x_evens(eo, J),
                    start=(eo == 0),
                    stop=(eo == NEB - 1),
                )
            nc.vector.tensor_copy(
                out=ktv[:, J * 2 * P : (J + 1) * 2 * P], in_=ps[:, : 2 * P]
            )
            # V natural: stationary x-block [e, s-tile], moving Wv e-tile.
            # Both k-tiles accumulate into one [P, 2, D] psum tile (still one
            # bank) so a single PSUM->SBUF copy moves the pair and the second
            # tile's matmuls never wait on the first tile's copy.
            vps = kv_ps.tile([P, 2, D], f32, tag="ps_v")
            for vi, lt in enumerate((2 * J, 2 * J + 1)):
                for eo in range(NEB):
                    nc.tensor.matmul(
                        vps[:, vi, :],
                        x_block(eo, J, vi),
                        w_sb["wv"][:, eo, :],
                        start=(eo == 0),
                        stop=(eo == NEB - 1),
                        skip_group_check=True,
                    )
            if vncopy == "dve":
                nc.vector.tensor_copy(
                    out=vn[:, 2 * J : 2 * J + 2, :], in_=vps[:]
                )
            else:
                nc.scalar.copy(out=vn[:, 2 * J : 2 * J + 2, :], in_=vps[:])
        def proj_chunk_q(J):
            # Q^T over all 4 positions of the chunk
            ps = proj_ps.tile([P, QBW], f32, tag="ps_q" if proj_split else "ps_kq")
            for eo in range(NEB):
                nc.tensor.matmul(
                    ps[:],
                    w_sb["wq"][:, eo, :],
                    x_all(eo, J),
                    start=(eo == 0),
                    stop=(eo == NEB - 1),
                )
            qv = qt.rearrange("p t q -> p (t q)")
            nc.vector.tensor_scalar(
                qv[:, J * QBW : (J + 1) * QBW],
                ps[:],
                SCALE,
                bq_sb[:],
                mybir.AluOpType.mult,
                mybir.AluOpType.add,
            )

        def proj_chunk(J):
            proj_chunk_kv(J)
            proj_chunk_q(J)

        def attention_tiles(J, order, pv, acc, idx0, nlt):
            """Emit score/exp/add/PV for the given k-tiles of block J.
            idx0 = number of tiles of this block already emitted."""
            for off, i in enumerate(order):
                idx = idx0 + off
                if evf:
                    # slot order [4J, 4J+2, 4J+3, 4J+1]: diagonal tile 2J+1
                    # sees exactly slots 1..2 -> contiguous [P, 3P)
                    lo, hi = (P, 3 * P) if i == 2 * J + 1 else (0, QBW)
                else:
                    lo, hi = (2 * P, QBW) if i == 2 * J + 1 else (0, QBW)
                sc = sc_ps.tile([P, QBW], f32, tag="sc")
                nc.tensor.matmul(
                    sc[:, lo:hi],
                    kt[:, i, :],
                    qt[:, 4 * J + lo // P : 4 * J + hi // P, :],
                    start=True,
                    stop=True,
                )
                if i == 2 * J and evf:
                    nc.vector.tensor_tensor(
                        out=sc[:, 0:QBW],
                        in0=sc[:, 0:QBW],
                        in1=maskw_sb[:],
                        op=mybir.AluOpType.add,
                    )
                elif i >= 2 * J:
                    m0 = lo if evf else (0 if i == 2 * J else 2 * P)
                    nc.vector.tensor_tensor(
                        out=sc[:, m0 : m0 + 2 * P],
                        in0=sc[:, m0 : m0 + 2 * P],
                        in1=mask_sb[:],
                        op=mybir.AluOpType.add,
                    )
                if idx == 0:
                    psrc = acc
                    nc.scalar.activation(
                        acc[:], sc[:], mybir.ActivationFunctionType.Exp
                    )
                else:
                    psrc = ppool.tile([P, QBW], f16, tag="p")
                    nc.scalar.activation(
                        psrc[:, lo:hi], sc[:, lo:hi],
                        mybir.ActivationFunctionType.Exp,
                    )
                    if adds == "mix":
                        add_eng = nc.gpsimd if lo else nc.vector
                    else:
                        add_eng = nc.vector if adds == "dve" else nc.gpsimd
                    add_eng.tensor_tensor(
                        out=acc[:, lo:hi],
                        in0=acc[:, lo:hi],
                        in1=psrc[:, lo:hi],
                        op=mybir.AluOpType.add,
                    )
                nc.tensor.matmul(
                    pv[:, lo:hi],
                    vn[:, i, :],
                    psrc[:, lo:hi],
                    start=(idx == 0),
                    stop=(idx == nlt - 1),
                    skip_group_check=True,
                )

        def attention_blk(J):
            """Block J: q-cols [512J, 512J+512), local k-tiles 0..2J+1."""
            nlt = 2 * J + 2
            pv = pv_ps.tile([P, QBW], f32, tag="pv")
            acc = apool.tile([P, QBW], f16, tag="acc")
            # emission order: full-width i=0 first (clears PSUM over the
            # whole block), then the masked tiles (their DVE-mask + exp
            # latency hides behind the remaining scores matmuls), ending on
            # a plain tile so only one exp latency is exposed at block end.
            if J == 0:
                order = [0, 1]
            elif masked_last:
                order = [0] + list(range(1, 2 * J)) + [2 * J, 2 * J + 1]
            elif order_variant == 1 and J >= 1:
                order = [0, 2 * J, 1, 2 * J + 1] + list(range(2, 2 * J))
                order = [t for t in order if t < 2 * J + 2][:2 * J + 2]
                if J == 1:
                    order = [0, 2, 3]
            elif order_variant == 2 and J >= 1:
                order = [2 * J, 0, 2 * J + 1] + list(range(1, 2 * J))
            else:
                order = [0, 2 * J, 2 * J + 1] + list(range(1, 2 * J))
            attention_tiles(J, order, pv, acc, 0, nlt)
            return pv, acc

        def finish_blk(J, pv, acc):
            """Block J tail: denominator matmul, staging copies, output DMA."""
            col0 = J * QBW
            sm = sum_ps.tile([1, QBW], f32, tag="sm")
            nc.tensor.matmul(sm[:], ones[:], acc[:], start=True, stop=True)
            if staging == "act":
                nc.scalar.copy(out=pvt_sb[:, col0 : col0 + QBW], in_=pv[:])
            else:
                nc.vector.tensor_copy(out=pvt_sb[:, col0 : col0 + QBW], in_=pv[:])
            if sum_eng == "act":
                nc.scalar.copy(out=sums_sb[:, col0 : col0 + QBW], in_=sm[:])
            else:
                nc.vector.tensor_copy(out=sums_sb[:, col0 : col0 + QBW], in_=sm[:])
            out_eng = {"pool": nc.gpsimd, "act": nc.scalar,
                       "sp": nc.sync}[out_ring]
            if out_merge and J < NQB - 1:
                # blocks 0..2 ship together once block 2 is staged
                if J == NQB - 2:
                    w = (NQB - 1) * QBW
                    out_eng.dma_start(pvt_d[:, :w], pvt_sb[:, :w])
                    out_eng.dma_start(sums_d[:, :w], sums_sb[:, :w])
                return
            out_eng.dma_start(
                pvt_d[:, col0 : col0 + QBW], pvt_sb[:, col0 : col0 + QBW]
            )
            out_eng.dma_start(
                sums_d[:, col0 : col0 + QBW], sums_sb[:, col0 : col0 + QBW]
            )

        pend = None
        pend_attn = None
        for _rep in range(reps):
            for J in range(NQB):
                load_chunk(J, _rep)
                if split_attn:
                    # Q first, then attention on the k-tiles that already
                    # exist (0..2J-1) BETWEEN Q-proj and K/V-proj, so the
                    # exp/add chains overlap the projection matmuls; the two
                    # new (masked, diagonal) tiles run after K/V lands.
                    proj_chunk_q(J)
                    if pend is not None:
                        finish_blk(*pend)
                        pend = None
                    nlt = 2 * J + 2
                    pv = pv_ps.tile([P, QBW], f32, tag="pv")
                    acc = apool.tile([P, QBW], f16, tag="acc")
                    part_a = list(range(0, max(0, 2 * J - 1)))
                    part_b = [2 * J, 2 * J + 1] + ([2 * J - 1] if J >= 1 else [])
                    attention_tiles(J, part_a, pv, acc, 0, nlt)
                    proj_chunk_kv(J)
                    attention_tiles(J, part_b, pv, acc, len(part_a), nlt)
                    if fin_delay:
                        pend = (J, pv, acc)
                    else:
                        finish_blk(J, pv, acc)
                    continue
                proj_chunk(J)
                if attn_delay:
                    if pend is not None:
                        finish_blk(*pend)
                        pend = None
                    if pend_attn is not None:
                        pend = (pend_attn, *attention_blk(pend_attn))
                    pend_attn = J
                    continue
                if not fin_delay:
                    finish_blk(J, *attention_blk(J))
                    continue
                if pend is not None:
                    finish_blk(*pend)
                pend = (J, *attention_blk(J))
        if pend is not None:
            finish_blk(*pend)
            pend = None
        if pend_attn is not None:
            finish_blk(pend_attn, *attention_blk(pend_attn))

    nc.compile()
    return nc


def _get_module(reps=1, **kw):
    key = ("nc", reps, tuple(sorted(kw.items())))
    if key not in _CACHE:
        _CACHE[key] = _build_module(reps, **kw)
    return _CACHE[key]


def _host_prep(x, Wq, bq, Wk, bk, Wv, bv, evf=False):
    """Build the 8 per-core input maps plus per-core q-column permutations.

    evf: evens-first slot order per chunk [4J, 4J+2, 4J+3, 4J+1] (own-parity
    tiles first so K/V projection can start after a half-chunk DMA; odds
    reversed so the diagonal tile 2J+1 sees contiguous slots 1..2)."""
    x = np.asarray(x, dtype=np.float32)
    tri = np.where(
        np.arange(P)[None, :] >= np.arange(P)[:, None], 0.0, NEG
    ).astype(np.float16)
    in_maps = []
    perms = []
    for c in range(8):
        b, h = divmod(c, 2)
        xt3 = np.ascontiguousarray(x[b].T).reshape(E, NT, P)
        if evf:
            perm4 = np.array([0, 2, 3, 1])
            slot_to_global = np.concatenate(
                [(4 * J + perm4) ^ h for J in range(NQB)]
            )
        else:
            slot_to_global = np.arange(NT) ^ h
        xt_perm = np.ascontiguousarray(
            xt3[:, slot_to_global, :].reshape(E, S)
        ).astype(np.float16)
        mask = np.concatenate(
            [tri, np.full((P, P), 0.0 if h == 0 else NEG, np.float16)], axis=1
        )
        m = {
            "xt": xt_perm,
            "wq": np.asarray(Wq, np.float16),
            "wk": np.asarray(Wk, np.float16),
            "wv": np.asarray(Wv, np.float16),
            "bq": np.asarray(bq, np.float32) * np.float32(SCALE),
            "mask": np.ascontiguousarray(mask),
            "ones": np.ones((P, 1), dtype=np.float16),
        }
        if evf:
            m["maskw"] = np.ascontiguousarray(np.concatenate(
                [tri, np.zeros((P, 2 * P), np.float16),
                 np.full((P, P), 0.0 if h == 0 else NEG, np.float16)], axis=1
            ))
        in_maps.append(m)
        # storage col -> global q row
        perm = np.empty(S, dtype=np.int64)
        for t in range(NT):
            perm[t * P : (t + 1) * P] = slot_to_global[t] * P + np.arange(P)
        perms.append(perm)
    return in_maps, perms


EVF = False  # evens-first layout (see _host_prep)


def kernel(x, Wq, bq, Wk, bk, Wv, bv):
    from concourse.bass_utils import run_bass_kernel_spmd

    nc = _get_module(evf=EVF)
    in_maps, perms = _host_prep(x, Wq, bq, Wk, bk, Wv, bv, evf=EVF)
    res = run_bass_kernel_spmd(
        nc,
        in_maps,
        core_ids=list(range(8)),
        trace=TRACE,
        **TRACE_KW,
    )
    _CACHE["last_result"] = res

    bv64 = np.asarray(bv, np.float64)
    out = np.empty((B, S, D), dtype=np.float32)
    for b in range(B):
        r0, r1 = res.results[2 * b], res.results[2 * b + 1]
        pv = np.zeros((D, S), dtype=np.float64)
        sm = np.zeros((S,), dtype=np.float64)
        for r, perm in ((r0, perms[2 * b]), (r1, perms[2 * b + 1])):
            pv[:, perm] += r["pvt"].astype(np.float64)
            sm[perm] += r["sums"][0].astype(np.float64)
        out[b] = ((pv / sm[None, :]).T + bv64[None, :]).astype(np.float32)
    return out
